# revision 13
# baseline (speedup 1.0000x reference)
"""Multi-head self-attention Trainium2 kernel (8 NeuronCores, SPMD).

Problem: B=2, S=2048, D=1024, H=16, Dk=64; torch-style Linear projections
(x @ W.T + b), custom softmax: p = exp(scores/8), attn = p / (sum(p) + 1e-8).

Sharding: 32 (batch, head) pairs over 8 cores -> core c handles batch c//4,
heads [4*(c%4), 4*(c%4)+4). Each core projects only its 256 features of
q/k/v; attention is embarrassingly parallel over (b, h).

v3 structure (single continuous pipeline, bf16 data everywhere off-PSUM):
  - qt arrives in bf16 (halved DMA), streamed in 4 s-chunks of 512; wk/wq +
    qt chunks 1/3 on the scalar HWDGE queue, qt chunks 0/2 + wv on sync,
    biases on gpsimd. ~16 dummy identity matmuls warm the PE's HAM clock
    gate during the DMA wait so the first projections run at 2.4GHz, not
    the 1.2GHz mid-pstate (measured 2x on the first ~7us of fills).
  - as soon as chunk 0 is projected (kT[0] tiles 0-3, qT[0] chunk 0), the
    scores/exp stream starts; v fills and later chunks interleave between
    steps, with AV matmuls lagging up to 3 steps behind their exp (pT pool
    bufs=4) so v production never gates the exp stream.
  - attention steady state is ACT(exp)-bound (~1.11us per t-step, one
    [128,1024] exp covers both heads of a pair via tile_position-packed
    scoresT in a 2-bank PSUM tile); measured steady exp gap 1127ns.
  - remaining projection work (qT[0] chunks 2-3, pair-1 qT/kT) runs as
    k-granular units (one 512-col matmul each, ~230ns) paced ~1.33/step in
    deadline order, so a fill never inserts a >2us bubble into the PE
    stream. Finalize pieces (PE transpose -> reciprocal -> out=ctx*r+bv)
    run 2-at-a-time between fills (never while a fill holds an x-slot:
    the shared 2-slot PSUM rotation would deadlock the in-order PE).
  - PSUM budget: scores 2x[128,1024] (8KB/part) + ctx 2x[65,512] (4KB) +
    2 rotating fill/transpose slots [128,512] (4KB) = 16KB = all 8 banks.

Output per core: [2048, 256] fp32 -> host concatenates features per batch.
"""

import sys

sys.path.insert(0, "/opt/trn_rl_repo")

from collections import deque
from contextlib import ExitStack

import numpy as np
import ml_dtypes

import concourse.bass as bass
import concourse.tile as tile
from concourse import bacc, mybir
from concourse.bass_utils import run_bass_kernel_spmd
from concourse.masks import make_identity

F32 = mybir.dt.float32
BF16 = mybir.dt.bfloat16

S = 2048  # sequence length
D = 1024  # d_model
J = 256  # features per core (4 heads x 64)
NKT = 8  # k-tiles of the d_model contraction
NSC = 4  # s-chunks of 512
NTT = 16  # t-tiles of 128
N_CORES = 8

_cached_nc = None
last_result = None  # BassKernelResults of the most recent run (for test.py)


def _build():
    nc = bacc.Bacc(None, target_bir_lowering=False)

    qt = nc.dram_tensor("qt", [D, S], BF16, kind="ExternalInput")
    wq = nc.dram_tensor("wq", [D, J], BF16, kind="ExternalInput")
    wk = nc.dram_tensor("wk", [D, J], BF16, kind="ExternalInput")
    wv = nc.dram_tensor("wv", [D, J], BF16, kind="ExternalInput")
    bq = nc.dram_tensor("bq", [J], F32, kind="ExternalInput")
    bk = nc.dram_tensor("bk", [J], F32, kind="ExternalInput")
    bv = nc.dram_tensor("bv", [J], F32, kind="ExternalInput")
    out = nc.dram_tensor("out", [S, J], F32, kind="ExternalOutput")

    with tile.TileContext(nc) as tc, ExitStack() as ctx:
        wts = ctx.enter_context(tc.tile_pool(name="wts", bufs=1))
        qtcp = ctx.enter_context(tc.tile_pool(name="qtc", bufs=1))
        qkp = ctx.enter_context(tc.tile_pool(name="qkp", bufs=1))
        vxp = ctx.enter_context(tc.tile_pool(name="vxp", bufs=1))
        bp = ctx.enter_context(tc.tile_pool(name="bp", bufs=1))
        cxp = ctx.enter_context(tc.tile_pool(name="cxp", bufs=16))
        pTp = ctx.enter_context(tc.tile_pool(name="pTp", bufs=4))
        outp = ctx.enter_context(tc.tile_pool(name="outp", bufs=1))
        rp = ctx.enter_context(tc.tile_pool(name="rp", bufs=8))
        pps = ctx.enter_context(tc.tile_pool(name="pps", bufs=1, space="PSUM"))

        # ---- DMA kickoff ----
        # scratch + identity first on gpsimd (gate the exp preload/warm-up)
        scratch = bp.tile([128, 1], F32, name="scratch")
        nc.gpsimd.memset(scratch[:], 0.0)
        ident = bp.tile([128, 128], BF16, name="ident")
        make_identity(nc, ident[:])

        # exp-table preload FIRST on ACT: a DMA_DIRECT2D on the scalar
        # engine retires only when its transfer completes, so any qt DMA
        # issued before the preload pushes the first real exp out ~10us
        nc.scalar.activation(
            scratch[:], scratch[:], mybir.ActivationFunctionType.Exp, scale=0.0
        )

        bq_t = bp.tile([128, 2], F32, name="bqt")
        bk_t = bp.tile([128, 2], F32, name="bkt")
        bv_t = bp.tile([128, J], F32, name="bvt")
        nc.gpsimd.dma_start(bq_t[:], bq.rearrange("(m p) -> p m", p=128))
        nc.gpsimd.dma_start(bk_t[:], bk.rearrange("(m p) -> p m", p=128))
        bvap = bv[:]
        bv_bcast = bass.AP(
            tensor=bvap.tensor, offset=bvap.offset, ap=[[0, 128], [1, J]]
        )
        nc.gpsimd.dma_start(bv_t[:], bv_bcast)

        # all bulk DMA rides the sync HWDGE queue in dependency order; the
        # scalar engine issues nothing (a DMA_DIRECT2D there would delay
        # the exp stream: it retires only when its transfer completes, and
        # the queue spin-up counts from the first issue)
        wk_t = wts.tile([128, NKT, J], BF16, name="wkt", tag="wkt")
        wq_t = wts.tile([128, NKT, J], BF16, name="wqt", tag="wqt")
        wv_t = wts.tile([128, NKT, J], BF16, name="wvt", tag="wvt")
        qt_r = qt.rearrange("(k p) s -> p k s", p=128)
        qtc = [
            qtcp.tile([128, NKT, 512], BF16, name=f"qtc{c}", tag=f"qtc{c}")
            for c in range(NSC)
        ]
        nc.sync.dma_start(wk_t[:], wk.rearrange("(k p) j -> p k j", p=128))
        nc.sync.dma_start(wq_t[:], wq.rearrange("(k p) j -> p k j", p=128))
        nc.sync.dma_start(qtc[0][:], qt_r[:, :, 0:512])
        nc.sync.dma_start(wv_t[:], wv.rearrange("(k p) j -> p k j", p=128))
        nc.sync.dma_start(qtc[1][:], qt_r[:, :, 512:1024])
        nc.sync.dma_start(qtc[2][:], qt_r[:, :, 1024:1536])
        nc.sync.dma_start(qtc[3][:], qt_r[:, :, 1536:2048])

        # Dummy matmuls keep the PE continuously busy from ~6.5us until the
        # first qt chunk lands (~13.5us), ramping the HAM clock gate to
        # 2.4GHz before the first projection fills. 16 was too few: the PE
        # idled >3.4us after them waiting on DMA and re-throttled to the
        # 1.2GHz mid-pstate (measured: first fills at 427ns/512col).
        warm = pps.tile([128, 128], F32, name="warm", tag="x0")
        for _ in range(52):
            nc.tensor.matmul(warm[:], ident[:], ident[:], start=True, stop=True)

        # Persistent projected tensors (all bf16)
        qT = [qkp.tile([128, S], BF16, name=f"qT{m}", tag=f"qT{m}") for m in range(2)]
        kT = [qkp.tile([128, S], BF16, name=f"kT{m}", tag=f"kT{m}") for m in range(2)]
        v_ext = []
        for t in range(NTT):
            vt = vxp.tile([128, 4, 65], BF16, name=f"vx{t}", tag=f"vx{t}")
            nc.gpsimd.memset(vt[:], 1.0)  # ones col [:, h, 64] survives
            v_ext.append(vt)
        out_tiles = [
            outp.tile([128, J], F32, name=f"ot{b}", tag=f"ot{b}") for b in range(16)
        ]

        # ---- fill units (projection work, runs in the x0/x1 PSUM slots) ----
        xflip = [0]

        def xtag():
            tag = f"x{xflip[0] % 2}"
            xflip[0] += 1
            return tag

        def fill_kq(kind, pair, c):
            """Project 512 s-cols of qT/kT for one head pair from qtc[c]."""
            for u in fill_kq_units(kind, pair, c):
                u()

        def fill_kq_units(kind, pair, c):
            w_t, dst, b_t = {
                "k": (wk_t, kT, bk_t),
                "q": (wq_t, qT, bq_t),
            }[kind]
            jsl = slice(pair * 128, pair * 128 + 128)
            cell = {}

            def mk(k):
                def f():
                    if k == 0:
                        cell["px"] = pps.tile(
                            [128, 512], F32, name=f"p{kind}", tag=xtag()
                        )
                    nc.tensor.matmul(
                        cell["px"][:],
                        w_t[:, k, jsl],
                        qtc[c][:, k, :],
                        start=(k == 0),
                        stop=(k == NKT - 1),
                    )
                return f

            def cp():
                s0 = c * 512
                nc.vector.tensor_scalar_add(
                    dst[pair][:, s0 : s0 + 512],
                    cell["px"][:],
                    b_t[:, pair : pair + 1],
                )

            return [mk(k) for k in range(NKT)] + [cp]

        def fill_v(c, half):
            """Project 2 t-tiles x all 4 heads of v from qtc[c] into v_ext."""
            px = pps.tile([128, 512], F32, name="pv", tag=xtag())
            for ii in range(2):
                i = 2 * half + ii
                isl = slice(i * 128, (i + 1) * 128)
                for k in range(NKT):
                    nc.tensor.matmul(
                        px[:, ii * 256 : (ii + 1) * 256],
                        qtc[c][:, k, isl],
                        wv_t[:, k, :],
                        start=(k == 0),
                        stop=(k == NKT - 1),
                    )
            for ii in range(2):
                i = 2 * half + ii
                nc.vector.tensor_copy(
                    v_ext[c * 4 + i][:, :, 0:64],
                    px[:, ii * 256 : (ii + 1) * 256].rearrange(
                        "p (h d) -> p h d", h=4
                    ),
                )

        # ---- finalize pieces ----
        pieces = deque()
        done_cnt = {}

        def piece(cs_tile, sc, h, i):
            def f():
                tp = pps.tile([128, 65], BF16, name="tp", tag=xtag())
                nc.tensor.transpose(
                    tp[:],
                    cs_tile[0:65, i * 128 : (i + 1) * 128],
                    ident[0:65, 0:65],
                )
                r = rp.tile([128, 1], F32, name="r", tag="r")
                nc.vector.reciprocal(r[:], tp[:, 64:65])
                blk = sc * 4 + i
                nc.vector.scalar_tensor_tensor(
                    out=out_tiles[blk][:, h * 64 : (h + 1) * 64],
                    in0=tp[:, 0:64],
                    scalar=r[:],
                    in1=bv_t[:, h * 64 : (h + 1) * 64],
                    op0=mybir.AluOpType.mult,
                    op1=mybir.AluOpType.add,
                )
                done_cnt[blk] = done_cnt.get(blk, 0) + 1
                if done_cnt[blk] == 4:
                    nc.sync.dma_start(
                        out[blk * 128 : (blk + 1) * 128, :], out_tiles[blk][:]
                    )
            return f

        # ---- attention pipeline (scores/exp decoupled from AV by <=3) ----
        blocks = [(p, sc) for p in range(2) for sc in range(NSC)]
        NB = len(blocks)
        ctx_ps = {}
        pts = {}

        def scores_exp(i):
            b, t = divmod(i, NTT)
            pair, sc = blocks[b]
            s0 = sc * 512
            qTt, kTt = qT[pair], kT[pair]
            tsl = slice(t * 128, (t + 1) * 128)
            g = pps.tile([128, 1024], F32, name="g", tag="grp", bufs=2)
            nc.tensor.matmul(
                g[:, 0:512],
                kTt[0:64, tsl],
                qTt[0:64, s0 : s0 + 512],
                start=True,
                stop=True,
                tile_position=(0, 0),
            )
            nc.tensor.matmul(
                g[:, 512:1024],
                kTt[64:128, tsl],
                qTt[64:128, s0 : s0 + 512],
                start=True,
                stop=True,
                tile_position=(64, 0),
            )
            pT_ = pTp.tile([128, 1024], BF16, name="pT_", tag="pT")
            nc.scalar.activation(
                pT_[:], g[:], mybir.ActivationFunctionType.Exp, scale=0.125
            )
            pts[i] = pT_

        def av(j):
            b, t = divmod(j, NTT)
            pair, sc = blocks[b]
            hA, hB = 2 * pair, 2 * pair + 1
            if t == 0:
                ctxA = pps.tile([65, 512], F32, name="ctxA", tag="ctx", bufs=2)
                ctxB = pps.tile([65, 512], F32, name="ctxB", tag="ctx", bufs=2)
                ctx_ps[b] = (ctxA, ctxB)
            ctxA, ctxB = ctx_ps[b]
            pT_ = pts.pop(j)
            st, sp = (t == 0), (t == NTT - 1)
            nc.tensor.matmul(
                ctxA[:], v_ext[t][:, hA, :], pT_[:, 0:512], start=st, stop=sp
            )
            nc.tensor.matmul(
                ctxB[:], v_ext[t][:, hB, :], pT_[:, 512:1024], start=st, stop=sp
            )
            if t == NTT - 1:
                del ctx_ps[b]
                csA = cxp.tile([65, 512], BF16, name="csA", tag="cs")
                nc.vector.tensor_copy(csA[:], ctxA[:])
                csB = cxp.tile([65, 512], BF16, name="csB", tag="cs")
                nc.vector.tensor_copy(csB[:], ctxB[:])
                for pi in range(4):
                    pieces.append(piece(csA, sc, hA, pi))
                    pieces.append(piece(csB, sc, hB, pi))

        # ---- ramp: stream chunks 0-3; v fills slot between early steps ----
        fill_kq("k", 0, 0)
        fill_kq("q", 0, 0)
        scores_exp(0)
        scores_exp(1)
        fill_v(0, 0)
        scores_exp(2)
        av(0)
        fill_v(0, 1)
        scores_exp(3)
        av(1)
        av(2)
        fill_kq("k", 0, 1)
        scores_exp(4)
        av(3)
        fill_v(1, 0)
        scores_exp(5)
        av(4)
        fill_v(1, 1)
        scores_exp(6)
        av(5)
        fill_kq("k", 0, 2)
        scores_exp(7)
        av(6)
        fill_v(2, 0)
        scores_exp(8)
        av(7)
        fill_v(2, 1)
        scores_exp(9)
        av(8)
        fill_kq("k", 0, 3)
        scores_exp(10)
        av(9)
        fill_v(3, 0)
        scores_exp(11)
        av(10)
        fill_v(3, 1)
        scores_exp(12)
        av(11)
        fill_kq("q", 0, 1)  # needed at step 16 (block 0,sc1)
        scores_exp(13)
        av(12)
        scores_exp(14)
        av(13)
        scores_exp(15)
        av(14)

        # ---- steady state: k-granular fills in deadline order ----
        units = deque()
        for kind, pair, c in [
            ("q", 0, 2),  # deadline step 32
            ("q", 0, 3),  # 48
            ("k", 1, 0),  # 64
            ("q", 1, 0),  # 64
            ("k", 1, 1),  # 68
            ("k", 1, 2),  # 72
            ("k", 1, 3),  # 76
            ("q", 1, 1),  # 80
            ("q", 1, 2),  # 96
            ("q", 1, 3),  # 112
        ]:
            units.extend(fill_kq_units(kind, pair, c))
            units.append("gap")  # piece window between fills

        for i in range(16, NB * NTT):
            scores_exp(i)
            av(i - 1)
            budget = 2 if i % 3 == 0 else 1
            for _ in range(budget):
                if units:
                    u = units.popleft()
                    if u == "gap":
                        for _ in range(2):
                            if pieces:
                                pieces.popleft()()
                    else:
                        u()
                elif pieces:
                    pieces.popleft()()
                    if len(pieces) > 8:
                        pieces.popleft()()
        av(NB * NTT - 1)
        while units:
            u = units.popleft()
            if u != "gap":
                u()
        while pieces:
            pieces.popleft()()

    nc.compile()
    return nc


def kernel(Q, Wq, bq, Wk, bk, Wv, bv):
    global _cached_nc, last_result
    Q = np.asarray(Q, dtype=np.float32)
    Wq, Wk, Wv = (np.asarray(w, dtype=np.float32) for w in (Wq, Wk, Wv))
    bq, bk, bv = (np.asarray(b, dtype=np.float32) for b in (bq, bk, bv))
    B = Q.shape[0]
    assert Q.shape == (B, S, D) and B * 4 == N_CORES

    if _cached_nc is None:
        _cached_nc = _build()
    nc = _cached_nc

    # host-side shard prep (bf16)
    bf = ml_dtypes.bfloat16
    qts = [np.ascontiguousarray(Q[b].T).astype(bf) for b in range(B)]
    wqs = [np.ascontiguousarray(Wq[g * J : (g + 1) * J, :].T).astype(bf) for g in range(4)]
    wks = [np.ascontiguousarray(Wk[g * J : (g + 1) * J, :].T).astype(bf) for g in range(4)]
    wvs = [np.ascontiguousarray(Wv[g * J : (g + 1) * J, :].T).astype(bf) for g in range(4)]

    in_maps = []
    for c in range(N_CORES):
        b, g = c // 4, c % 4
        jsl = slice(g * J, (g + 1) * J)
        in_maps.append(
            {
                "qt": qts[b],
                "wq": wqs[g],
                "wk": wks[g],
                "wv": wvs[g],
                "bq": np.ascontiguousarray(bq[jsl]),
                "bk": np.ascontiguousarray(bk[jsl]),
                "bv": np.ascontiguousarray(bv[jsl]),
            }
        )

    last_result = run_bass_kernel_spmd(nc, in_maps, list(range(N_CORES)))

    full = np.empty((B, S, D), dtype=np.float32)
    for c in range(N_CORES):
        b, g = c // 4, c % 4
        full[b, :, g * J : (g + 1) * J] = last_result.results[c]["out"]
    return full


# revision 16
# speedup vs baseline: 1.1698x; 1.1698x over previous
"""Multi-head self-attention Trainium2 kernel (8 NeuronCores, SPMD).

Problem: B=2, S=2048, D=1024, H=16, Dk=64; torch-style Linear projections
(x @ W.T + b), custom softmax: p = exp(scores/8), attn = p / (sum(p) + 1e-8).

Sharding: 32 (batch, head) pairs over 8 cores -> core c handles batch c//4,
heads [4*(c%4), 4*(c%4)+4). Each core projects only its 256 features of
q/k/v; attention is embarrassingly parallel over (b, h).

v3 structure (single continuous pipeline, bf16 data everywhere off-PSUM):
  - qt arrives in bf16 (halved DMA), streamed in 4 s-chunks of 512; wk/wq +
    qt chunks 1/3 on the scalar HWDGE queue, qt chunks 0/2 + wv on sync,
    biases on gpsimd. ~16 dummy identity matmuls warm the PE's HAM clock
    gate during the DMA wait so the first projections run at 2.4GHz, not
    the 1.2GHz mid-pstate (measured 2x on the first ~7us of fills).
  - as soon as chunk 0 is projected (kT[0] tiles 0-3, qT[0] chunk 0), the
    scores/exp stream starts; v fills and later chunks interleave between
    steps, with AV matmuls lagging up to 3 steps behind their exp (pT pool
    bufs=4) so v production never gates the exp stream.
  - attention steady state is ACT(exp)-bound (~1.11us per t-step, one
    [128,1024] exp covers both heads of a pair via tile_position-packed
    scoresT in a 2-bank PSUM tile); measured steady exp gap 1127ns.
  - remaining projection work (qT[0] chunks 2-3, pair-1 qT/kT) runs as
    k-granular units (one 512-col matmul each, ~230ns) paced ~1.33/step in
    deadline order, so a fill never inserts a >2us bubble into the PE
    stream. Finalize pieces (PE transpose -> reciprocal -> out=ctx*r+bv)
    run 2-at-a-time between fills (never while a fill holds an x-slot:
    the shared 2-slot PSUM rotation would deadlock the in-order PE).
  - PSUM budget: scores 2x[128,1024] (8KB/part) + ctx 2x[65,512] (4KB) +
    2 rotating fill/transpose slots [128,512] (4KB) = 16KB = all 8 banks.

Output per core: [2048, 256] fp32 -> host concatenates features per batch.
"""

import sys

sys.path.insert(0, "/opt/trn_rl_repo")

from collections import deque
from contextlib import ExitStack

import numpy as np
import ml_dtypes

import concourse.bass as bass
import concourse.tile as tile
from concourse import bacc, mybir
from concourse.bass_utils import run_bass_kernel_spmd
from concourse.masks import make_identity

F32 = mybir.dt.float32
F32R = mybir.dt.float32r
INT32 = mybir.dt.int32
BF16 = mybir.dt.bfloat16

S = 2048  # sequence length
D = 1024  # d_model
J = 256  # features per core (4 heads x 64)
NKT = 8  # k-tiles of the d_model contraction
NSC = 4  # s-chunks of 512
NTT = 16  # t-tiles of 128
N_CORES = 8

_cached_nc = None
last_result = None  # BassKernelResults of the most recent run (for test.py)


def _build():
    nc = bacc.Bacc(None, target_bir_lowering=False)

    qt = nc.dram_tensor("qt", [D, S], BF16, kind="ExternalInput")
    wq = nc.dram_tensor("wq", [D, J], BF16, kind="ExternalInput")
    wk = nc.dram_tensor("wk", [D, J], BF16, kind="ExternalInput")
    wv = nc.dram_tensor("wv", [D, J], BF16, kind="ExternalInput")
    bq = nc.dram_tensor("bq", [J], F32, kind="ExternalInput")
    bk = nc.dram_tensor("bk", [J], F32, kind="ExternalInput")
    bv = nc.dram_tensor("bv", [J], F32, kind="ExternalInput")
    out = nc.dram_tensor("out", [S, J], F32, kind="ExternalOutput")

    with tile.TileContext(nc) as tc, ExitStack() as ctx:
        wts = ctx.enter_context(tc.tile_pool(name="wts", bufs=1))
        qtcp = ctx.enter_context(tc.tile_pool(name="qtc", bufs=1))
        qkp = ctx.enter_context(tc.tile_pool(name="qkp", bufs=1))
        vxp = ctx.enter_context(tc.tile_pool(name="vxp", bufs=1))
        bp = ctx.enter_context(tc.tile_pool(name="bp", bufs=1))
        cxp = ctx.enter_context(tc.tile_pool(name="cxp", bufs=16))
        pTp = ctx.enter_context(tc.tile_pool(name="pTp", bufs=4))
        outp = ctx.enter_context(tc.tile_pool(name="outp", bufs=1))
        rp = ctx.enter_context(tc.tile_pool(name="rp", bufs=8))
        ytp = ctx.enter_context(tc.tile_pool(name="ytp", bufs=2))
        pps = ctx.enter_context(tc.tile_pool(name="pps", bufs=1, space="PSUM"))

        # ---- DMA kickoff ----
        # scratch + identity first on gpsimd (gate the exp preload/warm-up)
        scratch = bp.tile([128, 1], F32, name="scratch")
        nc.gpsimd.memset(scratch[:], 0.0)
        ident = bp.tile([128, 128], BF16, name="ident")
        make_identity(nc, ident[:])

        # exp-table preload FIRST on ACT: a DMA_DIRECT2D on the scalar
        # engine retires only when its transfer completes, so any qt DMA
        # issued before the preload pushes the first real exp out ~10us
        nc.scalar.activation(
            scratch[:], scratch[:], mybir.ActivationFunctionType.Exp, scale=0.0
        )

        bq_t = bp.tile([128, 2], F32, name="bqt")
        bk_t = bp.tile([128, 2], F32, name="bkt")
        bv_t = bp.tile([128, J], F32, name="bvt")
        nc.gpsimd.dma_start(bq_t[:], bq.rearrange("(m p) -> p m", p=128))
        nc.gpsimd.dma_start(bk_t[:], bk.rearrange("(m p) -> p m", p=128))
        bvap = bv[:]
        bv_bcast = bass.AP(
            tensor=bvap.tensor, offset=bvap.offset, ap=[[0, 128], [1, J]]
        )
        nc.gpsimd.dma_start(bv_t[:], bv_bcast)

        # all bulk DMA rides the sync HWDGE queue in dependency order; the
        # scalar engine issues nothing (a DMA_DIRECT2D there would delay
        # the exp stream: it retires only when its transfer completes, and
        # the queue spin-up counts from the first issue)
        wk_t = wts.tile([128, NKT, J], BF16, name="wkt", tag="wkt")
        wq_t = wts.tile([128, NKT, J], BF16, name="wqt", tag="wqt")
        wv_t = wts.tile([128, NKT, J], BF16, name="wvt", tag="wvt")
        qt_r = qt.rearrange("(k p) s -> p k s", p=128)
        qtc = [
            qtcp.tile([128, NKT, 512], BF16, name=f"qtc{c}", tag=f"qtc{c}")
            for c in range(NSC)
        ]
        nc.sync.dma_start(wk_t[:], wk.rearrange("(k p) j -> p k j", p=128))
        nc.sync.dma_start(qtc[0][:], qt_r[:, :, 0:512])
        nc.sync.dma_start(wq_t[:], wq.rearrange("(k p) j -> p k j", p=128))
        nc.sync.dma_start(wv_t[:], wv.rearrange("(k p) j -> p k j", p=128))
        nc.sync.dma_start(qtc[1][:], qt_r[:, :, 512:1024])
        nc.sync.dma_start(qtc[2][:], qt_r[:, :, 1024:1536])
        nc.sync.dma_start(qtc[3][:], qt_r[:, :, 1536:2048])

        # Dummy matmuls keep the PE continuously busy from ~6.5us until the
        # first qt chunk lands (~13.5us), ramping the HAM clock gate to
        # 2.4GHz before the first projection fills. 16 was too few: the PE
        # idled >3.4us after them waiting on DMA and re-throttled to the
        # 1.2GHz mid-pstate (measured: first fills at 427ns/512col).
        warm = pps.tile([128, 128], F32, name="warm", tag="x0")
        for _ in range(34):
            nc.tensor.matmul(warm[:], ident[:], ident[:], start=True, stop=True)

        # Persistent projected tensors (all bf16)
        qT = [qkp.tile([128, S], BF16, name=f"qT{m}", tag=f"qT{m}") for m in range(2)]
        kT = [qkp.tile([128, S], BF16, name=f"kT{m}", tag=f"kT{m}") for m in range(2)]
        v_ext = []
        for t in range(NTT):
            vt = vxp.tile([128, 4, 65], F32R, name=f"vx{t}", tag=f"vx{t}")
            nc.gpsimd.memset(vt[:].bitcast(F32), 1.0)  # ones col [:, h, 64] survives
            v_ext.append(vt)
        out_tiles = [
            outp.tile([128, J], F32, name=f"ot{b}", tag=f"ot{b}") for b in range(16)
        ]

        # ---- fill units (projection work, runs in the x0/x1 PSUM slots) ----
        xflip = [0]

        def xtag():
            tag = f"x{xflip[0] % 2}"
            xflip[0] += 1
            return tag

        def fill_kq(kind, pair, c):
            """Project 512 s-cols of qT/kT for one head pair from qtc[c]."""
            for u in fill_kq_units(kind, pair, c):
                u()

        def fill_kq_units(kind, pair, c):
            w_t, dst, b_t = {
                "k": (wk_t, kT, bk_t),
                "q": (wq_t, qT, bq_t),
            }[kind]
            jsl = slice(pair * 128, pair * 128 + 128)
            cell = {}

            def mk(k):
                def f():
                    if k == 0:
                        cell["px"] = pps.tile(
                            [128, 512], F32, name=f"p{kind}", tag=xtag()
                        )
                    nc.tensor.matmul(
                        cell["px"][:],
                        w_t[:, k, jsl],
                        qtc[c][:, k, :],
                        start=(k == 0),
                        stop=(k == NKT - 1),
                    )
                return f

            def cp():
                s0 = c * 512
                nc.vector.tensor_scalar_add(
                    dst[pair][:, s0 : s0 + 512],
                    cell["px"][:],
                    b_t[:, pair : pair + 1],
                )

            return [mk(k) for k in range(NKT)] + [cp]

        def fill_v(c, half):
            """Project 2 t-tiles x all 4 heads of v from qtc[c] into v_ext."""
            px = pps.tile([128, 512], F32, name="pv", tag=xtag())
            for ii in range(2):
                i = 2 * half + ii
                isl = slice(i * 128, (i + 1) * 128)
                for k in range(NKT):
                    nc.tensor.matmul(
                        px[:, ii * 256 : (ii + 1) * 256],
                        qtc[c][:, k, isl],
                        wv_t[:, k, :],
                        start=(k == 0),
                        stop=(k == NKT - 1),
                    )
            for ii in range(2):
                i = 2 * half + ii
                nc.vector.tensor_copy(
                    v_ext[c * 4 + i][:, :, 0:64],
                    px[:, ii * 256 : (ii + 1) * 256].rearrange(
                        "p (h d) -> p h d", h=4
                    ),
                )

        # ---- finalize pieces ----
        pieces = deque()
        done_cnt = {}

        def piece(cs_tile, sc, h, i):
            def f():
                tp = pps.tile([128, 65], BF16, name="tp", tag=xtag())
                nc.tensor.transpose(
                    tp[:],
                    cs_tile[0:65, i * 128 : (i + 1) * 128],
                    ident[0:65, 0:65],
                )
                r = rp.tile([128, 1], F32, name="r", tag="r")
                nc.vector.reciprocal(r[:], tp[:, 64:65])
                blk = sc * 4 + i
                nc.vector.scalar_tensor_tensor(
                    out=out_tiles[blk][:, h * 64 : (h + 1) * 64],
                    in0=tp[:, 0:64],
                    scalar=r[:],
                    in1=bv_t[:, h * 64 : (h + 1) * 64],
                    op0=mybir.AluOpType.mult,
                    op1=mybir.AluOpType.add,
                )
                done_cnt[blk] = done_cnt.get(blk, 0) + 1
                if done_cnt[blk] == 4:
                    nc.sync.dma_start(
                        out[blk * 128 : (blk + 1) * 128, :], out_tiles[blk][:]
                    )
            return f

        # ---- attention pipeline (scores/exp decoupled from AV by <=3) ----
        blocks = [(p, sc) for p in range(2) for sc in range(NSC)]
        NB = len(blocks)
        ctx_ps = {}
        pts = {}

        def scores_exp(i):
            b, t = divmod(i, NTT)
            pair, sc = blocks[b]
            s0 = sc * 512
            qTt, kTt = qT[pair], kT[pair]
            tsl = slice(t * 128, (t + 1) * 128)
            g = pps.tile([128, 1024], F32, name="g", tag="grp", bufs=2)
            nc.tensor.matmul(
                g[:, 0:512],
                kTt[0:64, tsl],
                qTt[0:64, s0 : s0 + 512],
                start=True,
                stop=True,
                tile_position=(0, 0),
            )
            nc.tensor.matmul(
                g[:, 512:1024],
                kTt[64:128, tsl],
                qTt[64:128, s0 : s0 + 512],
                start=True,
                stop=True,
                tile_position=(64, 0),
            )
            pT_ = pTp.tile([128, 1024], F32R, name="pT_", tag="pT")
            if False:
                # Schraudolph bit-trick exp on DVE: i32 = g*(2^23/(8 ln2)) +
                # (127<<23 - C + .5); the int32 bit pattern IS exp(g/8) to
                # ~3% (C=366392.3 balances the periodic mantissa error,
                # which the sum-normalized softmax then largely cancels).
                # Offloading every 4th steady step rebalances ACT vs DVE.
                yt = ytp.tile([128, 1024], F32, name="yt", tag="yt")
                nc.vector.tensor_scalar(
                    yt[:],
                    g[:],
                    1512775.3951133362,
                    1064986824.2,
                    mybir.AluOpType.mult,
                    mybir.AluOpType.add,
                )
                nc.vector.tensor_copy(pT_[:].bitcast(INT32), yt[:])
            else:
                nc.scalar.activation(
                    pT_[:], g[:], mybir.ActivationFunctionType.Exp, scale=0.125
                )
            pts[i] = pT_

        def av(j):
            b, t = divmod(j, NTT)
            pair, sc = blocks[b]
            hA, hB = 2 * pair, 2 * pair + 1
            if t == 0:
                ctxA = pps.tile([65, 512], F32, name="ctxA", tag="ctx", bufs=2)
                ctxB = pps.tile([65, 512], F32, name="ctxB", tag="ctx", bufs=2)
                ctx_ps[b] = (ctxA, ctxB)
            ctxA, ctxB = ctx_ps[b]
            pT_ = pts.pop(j)
            st, sp = (t == 0), (t == NTT - 1)
            nc.tensor.matmul(
                ctxA[:], v_ext[t][:, hA, :], pT_[:, 0:512], start=st, stop=sp
            )
            nc.tensor.matmul(
                ctxB[:], v_ext[t][:, hB, :], pT_[:, 512:1024], start=st, stop=sp
            )
            if t == NTT - 1:
                del ctx_ps[b]
                csA = cxp.tile([65, 512], BF16, name="csA", tag="cs")
                nc.vector.tensor_copy(csA[:], ctxA[:])
                csB = cxp.tile([65, 512], BF16, name="csB", tag="cs")
                nc.vector.tensor_copy(csB[:], ctxB[:])
                for pi in range(4):
                    pieces.append(piece(csA, sc, hA, pi))
                    pieces.append(piece(csB, sc, hB, pi))

        # ---- ramp: stream chunks 0-3; v fills slot between early steps ----
        fill_kq("k", 0, 0)
        fill_kq("q", 0, 0)
        scores_exp(0)
        scores_exp(1)
        fill_v(0, 0)
        scores_exp(2)
        av(0)
        fill_v(0, 1)
        scores_exp(3)
        av(1)
        av(2)
        fill_kq("k", 0, 1)
        scores_exp(4)
        av(3)
        fill_v(1, 0)
        scores_exp(5)
        av(4)
        fill_v(1, 1)
        scores_exp(6)
        av(5)
        fill_kq("k", 0, 2)
        scores_exp(7)
        av(6)
        fill_v(2, 0)
        scores_exp(8)
        av(7)
        fill_v(2, 1)
        scores_exp(9)
        av(8)
        fill_kq("k", 0, 3)
        scores_exp(10)
        av(9)
        fill_v(3, 0)
        scores_exp(11)
        av(10)
        fill_v(3, 1)
        scores_exp(12)
        av(11)
        fill_kq("q", 0, 1)  # needed at step 16 (block 0,sc1)
        scores_exp(13)
        av(12)
        scores_exp(14)
        av(13)
        scores_exp(15)
        av(14)

        # ---- steady state: k-granular fills in deadline order ----
        units = deque()
        for kind, pair, c in [
            ("q", 0, 2),  # deadline step 32
            ("q", 0, 3),  # 48
            ("k", 1, 0),  # 64
            ("q", 1, 0),  # 64
            ("k", 1, 1),  # 68
            ("k", 1, 2),  # 72
            ("k", 1, 3),  # 76
            ("q", 1, 1),  # 80
            ("q", 1, 2),  # 96
            ("q", 1, 3),  # 112
        ]:
            units.extend(fill_kq_units(kind, pair, c))
            units.append("gap")  # piece window between fills

        for i in range(16, NB * NTT):
            scores_exp(i)
            av(i - 1)
            budget = 2 if i % 3 == 0 else 1
            for _ in range(budget):
                if units:
                    u = units.popleft()
                    if u == "gap":
                        for _ in range(2):
                            if pieces:
                                pieces.popleft()()
                    else:
                        u()
                elif pieces:
                    pieces.popleft()()
                    if len(pieces) > 8:
                        pieces.popleft()()
        av(NB * NTT - 1)
        while units:
            u = units.popleft()
            if u != "gap":
                u()
        while pieces:
            pieces.popleft()()

    nc.compile()
    return nc


def kernel(Q, Wq, bq, Wk, bk, Wv, bv):
    global _cached_nc, last_result
    Q = np.asarray(Q, dtype=np.float32)
    Wq, Wk, Wv = (np.asarray(w, dtype=np.float32) for w in (Wq, Wk, Wv))
    bq, bk, bv = (np.asarray(b, dtype=np.float32) for b in (bq, bk, bv))
    B = Q.shape[0]
    assert Q.shape == (B, S, D) and B * 4 == N_CORES

    if _cached_nc is None:
        _cached_nc = _build()
    nc = _cached_nc

    # host-side shard prep (bf16)
    bf = ml_dtypes.bfloat16
    qts = [np.ascontiguousarray(Q[b].T).astype(bf) for b in range(B)]
    wqs = [np.ascontiguousarray(Wq[g * J : (g + 1) * J, :].T).astype(bf) for g in range(4)]
    wks = [np.ascontiguousarray(Wk[g * J : (g + 1) * J, :].T).astype(bf) for g in range(4)]
    wvs = [np.ascontiguousarray(Wv[g * J : (g + 1) * J, :].T).astype(bf) for g in range(4)]

    in_maps = []
    for c in range(N_CORES):
        b, g = c // 4, c % 4
        jsl = slice(g * J, (g + 1) * J)
        in_maps.append(
            {
                "qt": qts[b],
                "wq": wqs[g],
                "wk": wks[g],
                "wv": wvs[g],
                "bq": np.ascontiguousarray(bq[jsl]),
                "bk": np.ascontiguousarray(bk[jsl]),
                "bv": np.ascontiguousarray(bv[jsl]),
            }
        )

    last_result = run_bass_kernel_spmd(nc, in_maps, list(range(N_CORES)))

    full = np.empty((B, S, D), dtype=np.float32)
    for c in range(N_CORES):
        b, g = c // 4, c % 4
        full[b, :, g * J : (g + 1) * J] = last_result.results[c]["out"]
    return full


# revision 17
# speedup vs baseline: 1.1932x; 1.0200x over previous
"""Multi-head self-attention Trainium2 kernel (8 NeuronCores, SPMD).

Problem: B=2, S=2048, D=1024, H=16, Dk=64; torch-style Linear projections
(x @ W.T + b), custom softmax: p = exp(scores/8), attn = p / (sum(p) + 1e-8).

Sharding: 32 (batch, head) pairs over 8 cores -> core c handles batch c//4,
heads [4*(c%4), 4*(c%4)+4). Each core projects only its 256 features of
q/k/v; attention is embarrassingly parallel over (b, h).

v3 structure (single continuous pipeline, bf16 data everywhere off-PSUM):
  - qt arrives in bf16 (halved DMA), streamed in 4 s-chunks of 512; wk/wq +
    qt chunks 1/3 on the scalar HWDGE queue, qt chunks 0/2 + wv on sync,
    biases on gpsimd. ~16 dummy identity matmuls warm the PE's HAM clock
    gate during the DMA wait so the first projections run at 2.4GHz, not
    the 1.2GHz mid-pstate (measured 2x on the first ~7us of fills).
  - as soon as chunk 0 is projected (kT[0] tiles 0-3, qT[0] chunk 0), the
    scores/exp stream starts; v fills and later chunks interleave between
    steps, with AV matmuls lagging up to 3 steps behind their exp (pT pool
    bufs=4) so v production never gates the exp stream.
  - attention steady state is ACT(exp)-bound (~1.11us per t-step, one
    [128,1024] exp covers both heads of a pair via tile_position-packed
    scoresT in a 2-bank PSUM tile); measured steady exp gap 1127ns.
  - remaining projection work (qT[0] chunks 2-3, pair-1 qT/kT) runs as
    k-granular units (one 512-col matmul each, ~230ns) paced ~1.33/step in
    deadline order, so a fill never inserts a >2us bubble into the PE
    stream. Finalize pieces (PE transpose -> reciprocal -> out=ctx*r+bv)
    run 2-at-a-time between fills (never while a fill holds an x-slot:
    the shared 2-slot PSUM rotation would deadlock the in-order PE).
  - PSUM budget: scores 2x[128,1024] (8KB/part) + ctx 2x[65,512] (4KB) +
    2 rotating fill/transpose slots [128,512] (4KB) = 16KB = all 8 banks.

Output per core: [2048, 256] fp32 -> host concatenates features per batch.
"""

import sys

sys.path.insert(0, "/opt/trn_rl_repo")

from collections import deque
from contextlib import ExitStack

import numpy as np
import ml_dtypes

import concourse.bass as bass
import concourse.tile as tile
from concourse import bacc, mybir
from concourse.bass_utils import run_bass_kernel_spmd
from concourse.masks import make_identity

F32 = mybir.dt.float32
F32R = mybir.dt.float32r
INT32 = mybir.dt.int32
BF16 = mybir.dt.bfloat16

S = 2048  # sequence length
D = 1024  # d_model
J = 256  # features per core (4 heads x 64)
NKT = 8  # k-tiles of the d_model contraction
NSC = 4  # s-chunks of 512
NTT = 16  # t-tiles of 128
N_CORES = 8

_cached_nc = None
last_result = None  # BassKernelResults of the most recent run (for test.py)


def _build():
    nc = bacc.Bacc(None, target_bir_lowering=False)

    qt = nc.dram_tensor("qt", [D, S], BF16, kind="ExternalInput")
    wq = nc.dram_tensor("wq", [D, J], BF16, kind="ExternalInput")
    wk = nc.dram_tensor("wk", [D, J], BF16, kind="ExternalInput")
    wv = nc.dram_tensor("wv", [D, J], BF16, kind="ExternalInput")
    bq = nc.dram_tensor("bq", [J], F32, kind="ExternalInput")
    bk = nc.dram_tensor("bk", [J], F32, kind="ExternalInput")
    bv = nc.dram_tensor("bv", [J], F32, kind="ExternalInput")
    out = nc.dram_tensor("out", [S, J], F32, kind="ExternalOutput")

    with tile.TileContext(nc) as tc, ExitStack() as ctx:
        wts = ctx.enter_context(tc.tile_pool(name="wts", bufs=1))
        qtcp = ctx.enter_context(tc.tile_pool(name="qtc", bufs=1))
        qkp = ctx.enter_context(tc.tile_pool(name="qkp", bufs=1))
        vxp = ctx.enter_context(tc.tile_pool(name="vxp", bufs=1))
        bp = ctx.enter_context(tc.tile_pool(name="bp", bufs=1))
        cxp = ctx.enter_context(tc.tile_pool(name="cxp", bufs=16))
        pTp = ctx.enter_context(tc.tile_pool(name="pTp", bufs=4))
        outp = ctx.enter_context(tc.tile_pool(name="outp", bufs=1))
        rp = ctx.enter_context(tc.tile_pool(name="rp", bufs=8))
        ytp = ctx.enter_context(tc.tile_pool(name="ytp", bufs=2))
        pps = ctx.enter_context(tc.tile_pool(name="pps", bufs=1, space="PSUM"))

        # ---- DMA kickoff ----
        # scratch + identity first on gpsimd (gate the exp preload/warm-up)
        scratch = bp.tile([128, 1], F32, name="scratch")
        nc.gpsimd.memset(scratch[:], 0.0)
        ident = bp.tile([128, 128], BF16, name="ident")
        make_identity(nc, ident[:])

        # exp-table preload FIRST on ACT: a DMA_DIRECT2D on the scalar
        # engine retires only when its transfer completes, so any qt DMA
        # issued before the preload pushes the first real exp out ~10us
        nc.scalar.activation(
            scratch[:], scratch[:], mybir.ActivationFunctionType.Exp, scale=0.0
        )

        bq_t = bp.tile([128, 2], F32, name="bqt")
        bk_t = bp.tile([128, 2], F32, name="bkt")
        bv_t = bp.tile([128, J], F32, name="bvt")
        nc.gpsimd.dma_start(bq_t[:], bq.rearrange("(m p) -> p m", p=128))
        nc.gpsimd.dma_start(bk_t[:], bk.rearrange("(m p) -> p m", p=128))
        bvap = bv[:]
        bv_bcast = bass.AP(
            tensor=bvap.tensor, offset=bvap.offset, ap=[[0, 128], [1, J]]
        )
        nc.gpsimd.dma_start(bv_t[:], bv_bcast)

        # all bulk DMA rides the sync HWDGE queue in dependency order; the
        # scalar engine issues nothing (a DMA_DIRECT2D there would delay
        # the exp stream: it retires only when its transfer completes, and
        # the queue spin-up counts from the first issue)
        wk_t = wts.tile([128, NKT, J], BF16, name="wkt", tag="wkt")
        wq_t = wts.tile([128, NKT, J], BF16, name="wqt", tag="wqt")
        wv_t = wts.tile([128, NKT, J], BF16, name="wvt", tag="wvt")
        qt_r = qt.rearrange("(k p) s -> p k s", p=128)
        qtc = [
            qtcp.tile([128, NKT, 512], BF16, name=f"qtc{c}", tag=f"qtc{c}")
            for c in range(NSC)
        ]
        nc.sync.dma_start(wk_t[:], wk.rearrange("(k p) j -> p k j", p=128))
        nc.sync.dma_start(qtc[0][:], qt_r[:, :, 0:512])
        nc.sync.dma_start(wq_t[:], wq.rearrange("(k p) j -> p k j", p=128))
        nc.sync.dma_start(wv_t[:], wv.rearrange("(k p) j -> p k j", p=128))
        nc.sync.dma_start(qtc[1][:], qt_r[:, :, 512:1024])
        nc.sync.dma_start(qtc[2][:], qt_r[:, :, 1024:1536])
        nc.sync.dma_start(qtc[3][:], qt_r[:, :, 1536:2048])

        # Dummy matmuls keep the PE continuously busy from ~6.5us until the
        # first qt chunk lands (~13.5us), ramping the HAM clock gate to
        # 2.4GHz before the first projection fills. 16 was too few: the PE
        # idled >3.4us after them waiting on DMA and re-throttled to the
        # 1.2GHz mid-pstate (measured: first fills at 427ns/512col).
        warm = pps.tile([128, 128], F32, name="warm", tag="x0")
        for _ in range(16):
            nc.tensor.matmul(warm[:], ident[:], ident[:], start=True, stop=True)

        # Persistent projected tensors (all bf16)
        qT = [qkp.tile([128, S], BF16, name=f"qT{m}", tag=f"qT{m}") for m in range(2)]
        kT = [qkp.tile([128, S], BF16, name=f"kT{m}", tag=f"kT{m}") for m in range(2)]
        v_ext = []
        for t in range(NTT):
            vt = vxp.tile([128, 4, 65], BF16, name=f"vx{t}", tag=f"vx{t}")
            nc.gpsimd.memset(vt[:], 1.0)  # ones col [:, h, 64] survives
            v_ext.append(vt)
        out_tiles = [
            outp.tile([128, J], F32, name=f"ot{b}", tag=f"ot{b}") for b in range(16)
        ]

        # ---- fill units (projection work, runs in the x0/x1 PSUM slots) ----
        xflip = [0]

        def xtag():
            tag = f"x{xflip[0] % 2}"
            xflip[0] += 1
            return tag

        def fill_kq(kind, pair, c):
            """Project 512 s-cols of qT/kT for one head pair from qtc[c]."""
            for u in fill_kq_units(kind, pair, c):
                u()

        def fill_kq_units(kind, pair, c):
            w_t, dst, b_t = {
                "k": (wk_t, kT, bk_t),
                "q": (wq_t, qT, bq_t),
            }[kind]
            jsl = slice(pair * 128, pair * 128 + 128)
            cell = {}

            def mk(k):
                def f():
                    if k == 0:
                        cell["px"] = pps.tile(
                            [128, 512], F32, name=f"p{kind}", tag=xtag()
                        )
                    nc.tensor.matmul(
                        cell["px"][:],
                        w_t[:, k, jsl],
                        qtc[c][:, k, :],
                        start=(k == 0),
                        stop=(k == NKT - 1),
                    )
                return f

            def cp():
                s0 = c * 512
                nc.vector.tensor_scalar_add(
                    dst[pair][:, s0 : s0 + 512],
                    cell["px"][:],
                    b_t[:, pair : pair + 1],
                )

            return [mk(k) for k in range(NKT)] + [cp]

        def fill_v(c, half):
            """Project 2 t-tiles x all 4 heads of v from qtc[c] into v_ext."""
            px = pps.tile([128, 512], F32, name="pv", tag=xtag())
            for ii in range(2):
                i = 2 * half + ii
                isl = slice(i * 128, (i + 1) * 128)
                for k in range(NKT):
                    nc.tensor.matmul(
                        px[:, ii * 256 : (ii + 1) * 256],
                        qtc[c][:, k, isl],
                        wv_t[:, k, :],
                        start=(k == 0),
                        stop=(k == NKT - 1),
                    )
            for ii in range(2):
                i = 2 * half + ii
                nc.vector.tensor_copy(
                    v_ext[c * 4 + i][:, :, 0:64],
                    px[:, ii * 256 : (ii + 1) * 256].rearrange(
                        "p (h d) -> p h d", h=4
                    ),
                )

        # ---- finalize pieces ----
        pieces = deque()
        done_cnt = {}

        def piece(cs_tile, sc, h, i):
            def f():
                tp = pps.tile([128, 65], BF16, name="tp", tag=xtag())
                nc.tensor.transpose(
                    tp[:],
                    cs_tile[0:65, i * 128 : (i + 1) * 128],
                    ident[0:65, 0:65],
                )
                r = rp.tile([128, 1], F32, name="r", tag="r")
                nc.vector.reciprocal(r[:], tp[:, 64:65])
                blk = sc * 4 + i
                nc.vector.scalar_tensor_tensor(
                    out=out_tiles[blk][:, h * 64 : (h + 1) * 64],
                    in0=tp[:, 0:64],
                    scalar=r[:],
                    in1=bv_t[:, h * 64 : (h + 1) * 64],
                    op0=mybir.AluOpType.mult,
                    op1=mybir.AluOpType.add,
                )
                done_cnt[blk] = done_cnt.get(blk, 0) + 1
                if done_cnt[blk] == 4:
                    nc.sync.dma_start(
                        out[blk * 128 : (blk + 1) * 128, :], out_tiles[blk][:]
                    )
            return f

        # ---- attention pipeline (scores/exp decoupled from AV by <=3) ----
        blocks = [(p, sc) for p in range(2) for sc in range(NSC)]
        NB = len(blocks)
        ctx_ps = {}
        pts = {}

        def scores_exp(i):
            b, t = divmod(i, NTT)
            pair, sc = blocks[b]
            s0 = sc * 512
            qTt, kTt = qT[pair], kT[pair]
            tsl = slice(t * 128, (t + 1) * 128)
            g = pps.tile([128, 1024], F32, name="g", tag="grp", bufs=2)
            nc.tensor.matmul(
                g[:, 0:512],
                kTt[0:64, tsl],
                qTt[0:64, s0 : s0 + 512],
                start=True,
                stop=True,
                tile_position=(0, 0),
            )
            nc.tensor.matmul(
                g[:, 512:1024],
                kTt[64:128, tsl],
                qTt[64:128, s0 : s0 + 512],
                start=True,
                stop=True,
                tile_position=(64, 0),
            )
            pT_ = pTp.tile([128, 1024], BF16, name="pT_", tag="pT")
            nc.scalar.activation(
                pT_[:], g[:], mybir.ActivationFunctionType.Exp, scale=0.125
            )
            pts[i] = pT_

        def av(j):
            b, t = divmod(j, NTT)
            pair, sc = blocks[b]
            hA, hB = 2 * pair, 2 * pair + 1
            if t == 0:
                ctxA = pps.tile([65, 512], F32, name="ctxA", tag="ctx", bufs=2)
                ctxB = pps.tile([65, 512], F32, name="ctxB", tag="ctx", bufs=2)
                ctx_ps[b] = (ctxA, ctxB)
            ctxA, ctxB = ctx_ps[b]
            pT_ = pts.pop(j)
            st, sp = (t == 0), (t == NTT - 1)
            nc.tensor.matmul(
                ctxA[:], v_ext[t][:, hA, :], pT_[:, 0:512], start=st, stop=sp
            )
            nc.tensor.matmul(
                ctxB[:], v_ext[t][:, hB, :], pT_[:, 512:1024], start=st, stop=sp
            )
            if t == NTT - 1:
                del ctx_ps[b]
                csA = cxp.tile([65, 512], BF16, name="csA", tag="cs")
                nc.vector.tensor_copy(csA[:], ctxA[:])
                csB = cxp.tile([65, 512], BF16, name="csB", tag="cs")
                nc.vector.tensor_copy(csB[:], ctxB[:])
                for pi in range(4):
                    pieces.append(piece(csA, sc, hA, pi))
                    pieces.append(piece(csB, sc, hB, pi))

        # ---- ramp: stream chunks 0-3; v fills slot between early steps ----
        fill_kq("k", 0, 0)
        fill_kq("q", 0, 0)
        scores_exp(0)
        scores_exp(1)
        fill_v(0, 0)
        scores_exp(2)
        av(0)
        fill_v(0, 1)
        scores_exp(3)
        av(1)
        av(2)
        fill_kq("k", 0, 1)
        scores_exp(4)
        av(3)
        fill_v(1, 0)
        scores_exp(5)
        av(4)
        fill_v(1, 1)
        scores_exp(6)
        av(5)
        fill_kq("k", 0, 2)
        scores_exp(7)
        av(6)
        fill_v(2, 0)
        scores_exp(8)
        av(7)
        fill_v(2, 1)
        scores_exp(9)
        av(8)
        fill_kq("k", 0, 3)
        scores_exp(10)
        av(9)
        fill_v(3, 0)
        scores_exp(11)
        av(10)
        fill_v(3, 1)
        scores_exp(12)
        av(11)
        fill_kq("q", 0, 1)  # needed at step 16 (block 0,sc1)
        scores_exp(13)
        av(12)
        scores_exp(14)
        av(13)
        scores_exp(15)
        av(14)

        # ---- steady state: k-granular fills in deadline order ----
        units = deque()
        for kind, pair, c in [
            ("q", 0, 2),  # deadline step 32
            ("q", 0, 3),  # 48
            ("k", 1, 0),  # 64
            ("q", 1, 0),  # 64
            ("k", 1, 1),  # 68
            ("k", 1, 2),  # 72
            ("k", 1, 3),  # 76
            ("q", 1, 1),  # 80
            ("q", 1, 2),  # 96
            ("q", 1, 3),  # 112
        ]:
            units.extend(fill_kq_units(kind, pair, c))
            units.append("gap")  # piece window between fills

        for i in range(16, NB * NTT):
            scores_exp(i)
            av(i - 1)
            budget = 2 if i % 3 == 0 else 1
            for _ in range(budget):
                if units:
                    u = units.popleft()
                    if u == "gap":
                        for _ in range(2):
                            if pieces:
                                pieces.popleft()()
                    else:
                        u()
                elif pieces:
                    pieces.popleft()()
                    if len(pieces) > 8:
                        pieces.popleft()()
        av(NB * NTT - 1)
        while units:
            u = units.popleft()
            if u != "gap":
                u()
        while pieces:
            pieces.popleft()()

    nc.compile()
    return nc


def kernel(Q, Wq, bq, Wk, bk, Wv, bv):
    global _cached_nc, last_result
    Q = np.asarray(Q, dtype=np.float32)
    Wq, Wk, Wv = (np.asarray(w, dtype=np.float32) for w in (Wq, Wk, Wv))
    bq, bk, bv = (np.asarray(b, dtype=np.float32) for b in (bq, bk, bv))
    B = Q.shape[0]
    assert Q.shape == (B, S, D) and B * 4 == N_CORES

    if _cached_nc is None:
        _cached_nc = _build()
    nc = _cached_nc

    # host-side shard prep (bf16)
    bf = ml_dtypes.bfloat16
    qts = [np.ascontiguousarray(Q[b].T).astype(bf) for b in range(B)]
    wqs = [np.ascontiguousarray(Wq[g * J : (g + 1) * J, :].T).astype(bf) for g in range(4)]
    wks = [np.ascontiguousarray(Wk[g * J : (g + 1) * J, :].T).astype(bf) for g in range(4)]
    wvs = [np.ascontiguousarray(Wv[g * J : (g + 1) * J, :].T).astype(bf) for g in range(4)]

    in_maps = []
    for c in range(N_CORES):
        b, g = c // 4, c % 4
        jsl = slice(g * J, (g + 1) * J)
        in_maps.append(
            {
                "qt": qts[b],
                "wq": wqs[g],
                "wk": wks[g],
                "wv": wvs[g],
                "bq": np.ascontiguousarray(bq[jsl]),
                "bk": np.ascontiguousarray(bk[jsl]),
                "bv": np.ascontiguousarray(bv[jsl]),
            }
        )

    last_result = run_bass_kernel_spmd(nc, in_maps, list(range(N_CORES)))

    full = np.empty((B, S, D), dtype=np.float32)
    for c in range(N_CORES):
        b, g = c // 4, c % 4
        full[b, :, g * J : (g + 1) * J] = last_result.results[c]["out"]
    return full


# revision 19
# speedup vs baseline: 1.2017x; 1.0071x over previous
"""Multi-head self-attention Trainium2 kernel (8 NeuronCores, SPMD).

Problem: B=2, S=2048, D=1024, H=16, Dk=64; torch-style Linear projections
(x @ W.T + b), custom softmax: p = exp(scores/8), attn = p / (sum(p) + 1e-8).

Sharding: 32 (batch, head) pairs over 8 cores -> core c handles batch c//4,
heads [4*(c%4), 4*(c%4)+4). Each core projects only its 256 features of
q/k/v; attention is embarrassingly parallel over (b, h).

v3 structure (single continuous pipeline, bf16 data everywhere off-PSUM):
  - qt arrives in bf16 (halved DMA), streamed in 4 s-chunks of 512; all bulk
    DMA rides the sync HWDGE queue in dependency order (wk, qt0, wq, wv,
    qt1-3), biases on gpsimd, and the scalar engine issues NO DMA (its
    DMA_DIRECT2D retires only on transfer completion, delaying the exp
    stream ~10us). 16 dummy identity matmuls warm the PE's HAM clock gate
    during the DMA wait so the first projections run at 2.4GHz, not the
    1.2GHz mid-pstate (measured 2x on the first ~7us of fills).
  - as soon as chunk 0 is projected (kT[0] tiles 0-3, qT[0] chunk 0), the
    scores/exp stream starts; v fills and later chunks interleave between
    steps, with AV matmuls lagging up to 3 steps behind their exp (pT pool
    bufs=4) so v production never gates the exp stream.
  - attention steady state is ACT(exp)-bound (~1.11us per t-step, one
    [128,1024] exp covers both heads of a pair via tile_position-packed
    scoresT in a 2-bank PSUM tile); measured steady exp gap 1127ns.
  - remaining projection work (qT[0] chunks 2-3, pair-1 qT/kT) runs as
    k-granular units (one 512-col matmul each, ~230ns) paced ~1.33/step in
    deadline order, so a fill never inserts a >2us bubble into the PE
    stream. Finalize pieces (PE transpose -> reciprocal -> out=ctx*r+bv)
    run 2-at-a-time between fills (never while a fill holds an x-slot:
    the shared 2-slot PSUM rotation would deadlock the in-order PE).
  - PSUM budget: scores 2x[128,1024] (8KB/part) + ctx 2x[65,512] (4KB) +
    2 rotating fill/transpose slots [128,512] (4KB) = 16KB = all 8 banks.

Output per core: [2048, 256] fp32 -> host concatenates features per batch.
"""

import sys

sys.path.insert(0, "/opt/trn_rl_repo")

from collections import deque
from contextlib import ExitStack

import numpy as np
import ml_dtypes

import concourse.bass as bass
import concourse.tile as tile
from concourse import bacc, mybir
from concourse.bass_utils import run_bass_kernel_spmd
from concourse.masks import make_identity

F32 = mybir.dt.float32
F32R = mybir.dt.float32r
INT32 = mybir.dt.int32
BF16 = mybir.dt.bfloat16

S = 2048  # sequence length
D = 1024  # d_model
J = 256  # features per core (4 heads x 64)
NKT = 8  # k-tiles of the d_model contraction
NSC = 4  # s-chunks of 512
NTT = 16  # t-tiles of 128
N_CORES = 8

_cached_nc = None
last_result = None  # BassKernelResults of the most recent run (for test.py)


def _build():
    nc = bacc.Bacc(None, target_bir_lowering=False)

    qt = nc.dram_tensor("qt", [D, S], BF16, kind="ExternalInput")
    wq = nc.dram_tensor("wq", [D, J], BF16, kind="ExternalInput")
    wk = nc.dram_tensor("wk", [D, J], BF16, kind="ExternalInput")
    wv = nc.dram_tensor("wv", [D, J], BF16, kind="ExternalInput")
    bq = nc.dram_tensor("bq", [J], F32, kind="ExternalInput")
    bk = nc.dram_tensor("bk", [J], F32, kind="ExternalInput")
    bv = nc.dram_tensor("bv", [J], F32, kind="ExternalInput")
    out = nc.dram_tensor("out", [S, J], F32, kind="ExternalOutput")

    with tile.TileContext(nc) as tc, ExitStack() as ctx:
        wts = ctx.enter_context(tc.tile_pool(name="wts", bufs=1))
        qtcp = ctx.enter_context(tc.tile_pool(name="qtc", bufs=1))
        qkp = ctx.enter_context(tc.tile_pool(name="qkp", bufs=1))
        vxp = ctx.enter_context(tc.tile_pool(name="vxp", bufs=1))
        bp = ctx.enter_context(tc.tile_pool(name="bp", bufs=1))
        cxp = ctx.enter_context(tc.tile_pool(name="cxp", bufs=16))
        pTp = ctx.enter_context(tc.tile_pool(name="pTp", bufs=4))
        outp = ctx.enter_context(tc.tile_pool(name="outp", bufs=1))
        rp = ctx.enter_context(tc.tile_pool(name="rp", bufs=8))
        ytp = ctx.enter_context(tc.tile_pool(name="ytp", bufs=2))
        pps = ctx.enter_context(tc.tile_pool(name="pps", bufs=1, space="PSUM"))

        # ---- DMA kickoff ----
        # scratch + identity first on gpsimd (gate the exp preload/warm-up)
        scratch = bp.tile([128, 1], F32, name="scratch")
        nc.gpsimd.memset(scratch[:], 0.0)
        ident = bp.tile([128, 128], BF16, name="ident")
        make_identity(nc, ident[:])

        # exp-table preload FIRST on ACT: a DMA_DIRECT2D on the scalar
        # engine retires only when its transfer completes, so any qt DMA
        # issued before the preload pushes the first real exp out ~10us
        nc.scalar.activation(
            scratch[:], scratch[:], mybir.ActivationFunctionType.Exp, scale=0.0
        )

        bq_t = bp.tile([128, 2], F32, name="bqt")
        bk_t = bp.tile([128, 2], F32, name="bkt")
        bv_t = bp.tile([128, J], F32, name="bvt")
        nc.gpsimd.dma_start(bq_t[:], bq.rearrange("(m p) -> p m", p=128))
        nc.gpsimd.dma_start(bk_t[:], bk.rearrange("(m p) -> p m", p=128))
        bvap = bv[:]
        bv_bcast = bass.AP(
            tensor=bvap.tensor, offset=bvap.offset, ap=[[0, 128], [1, J]]
        )
        nc.gpsimd.dma_start(bv_t[:], bv_bcast)

        # all bulk DMA rides the sync HWDGE queue in dependency order; the
        # scalar engine issues nothing (a DMA_DIRECT2D there would delay
        # the exp stream: it retires only when its transfer completes, and
        # the queue spin-up counts from the first issue)
        wk_t = wts.tile([128, NKT, J], BF16, name="wkt", tag="wkt")
        wq_t = wts.tile([128, NKT, J], BF16, name="wqt", tag="wqt")
        wv_t = wts.tile([128, NKT, J], BF16, name="wvt", tag="wvt")
        qt_r = qt.rearrange("(k p) s -> p k s", p=128)
        qtc = [
            qtcp.tile([128, NKT, 512], BF16, name=f"qtc{c}", tag=f"qtc{c}")
            for c in range(NSC)
        ]
        nc.sync.dma_start(wk_t[:], wk.rearrange("(k p) j -> p k j", p=128))
        nc.sync.dma_start(qtc[0][:], qt_r[:, :, 0:512])
        nc.sync.dma_start(wq_t[:], wq.rearrange("(k p) j -> p k j", p=128))
        nc.sync.dma_start(wv_t[:], wv.rearrange("(k p) j -> p k j", p=128))
        nc.sync.dma_start(qtc[1][:], qt_r[:, :, 512:1024])
        nc.sync.dma_start(qtc[2][:], qt_r[:, :, 1024:1536])
        nc.sync.dma_start(qtc[3][:], qt_r[:, :, 1536:2048])

        # Dummy matmuls keep the PE continuously busy from ~6.5us until
        # qtc0 lands (~13.7us), ramping the HAM clock gate to 2.4GHz before
        # the first projection fills. 16 ended at ~10.1us and the >3.4us
        # idle until the DMA-gated first fill re-throttled the PE to the
        # 1.2GHz mid-pstate (measured: 634ns/512col fills, ~6us lost).
        warm = pps.tile([128, 128], F32, name="warm", tag="x0")
        for _ in range(52):
            nc.tensor.matmul(warm[:], ident[:], ident[:], start=True, stop=True)

        # Persistent projected tensors (all bf16)
        qT = [qkp.tile([128, S], BF16, name=f"qT{m}", tag=f"qT{m}") for m in range(2)]
        kT = [qkp.tile([128, S], BF16, name=f"kT{m}", tag=f"kT{m}") for m in range(2)]
        v_ext = []
        for t in range(NTT):
            vt = vxp.tile([128, 4, 65], BF16, name=f"vx{t}", tag=f"vx{t}")
            nc.gpsimd.memset(vt[:], 1.0)  # ones col [:, h, 64] survives
            v_ext.append(vt)
        out_tiles = [
            outp.tile([128, J], F32, name=f"ot{b}", tag=f"ot{b}") for b in range(16)
        ]

        # ---- fill units (projection work, runs in the x0/x1 PSUM slots) ----
        xflip = [0]

        def xtag():
            tag = f"x{xflip[0] % 2}"
            xflip[0] += 1
            return tag

        def fill_kq(kind, pair, c):
            """Project 512 s-cols of qT/kT for one head pair from qtc[c]."""
            for u in fill_kq_units(kind, pair, c):
                u()

        def fill_kq_units(kind, pair, c):
            w_t, dst, b_t = {
                "k": (wk_t, kT, bk_t),
                "q": (wq_t, qT, bq_t),
            }[kind]
            jsl = slice(pair * 128, pair * 128 + 128)
            cell = {}

            def mk(k):
                def f():
                    if k == 0:
                        cell["px"] = pps.tile(
                            [128, 512], F32, name=f"p{kind}", tag=xtag()
                        )
                    nc.tensor.matmul(
                        cell["px"][:],
                        w_t[:, k, jsl],
                        qtc[c][:, k, :],
                        start=(k == 0),
                        stop=(k == NKT - 1),
                    )
                return f

            def cp():
                s0 = c * 512
                nc.vector.tensor_scalar_add(
                    dst[pair][:, s0 : s0 + 512],
                    cell["px"][:],
                    b_t[:, pair : pair + 1],
                )

            return [mk(k) for k in range(NKT)] + [cp]

        def fill_v(c, half):
            """Project 2 t-tiles x all 4 heads of v from qtc[c] into v_ext."""
            px = pps.tile([128, 512], F32, name="pv", tag=xtag())
            for ii in range(2):
                i = 2 * half + ii
                isl = slice(i * 128, (i + 1) * 128)
                for k in range(NKT):
                    nc.tensor.matmul(
                        px[:, ii * 256 : (ii + 1) * 256],
                        qtc[c][:, k, isl],
                        wv_t[:, k, :],
                        start=(k == 0),
                        stop=(k == NKT - 1),
                    )
            for ii in range(2):
                i = 2 * half + ii
                nc.vector.tensor_copy(
                    v_ext[c * 4 + i][:, :, 0:64],
                    px[:, ii * 256 : (ii + 1) * 256].rearrange(
                        "p (h d) -> p h d", h=4
                    ),
                )

        # ---- finalize pieces ----
        pieces = deque()
        done_cnt = {}

        def piece(cs_tile, sc, h, i):
            def f():
                tp = pps.tile([128, 65], BF16, name="tp", tag=xtag())
                nc.tensor.transpose(
                    tp[:],
                    cs_tile[0:65, i * 128 : (i + 1) * 128],
                    ident[0:65, 0:65],
                )
                r = rp.tile([128, 1], F32, name="r", tag="r")
                nc.vector.reciprocal(r[:], tp[:, 64:65])
                blk = sc * 4 + i
                nc.vector.scalar_tensor_tensor(
                    out=out_tiles[blk][:, h * 64 : (h + 1) * 64],
                    in0=tp[:, 0:64],
                    scalar=r[:],
                    in1=bv_t[:, h * 64 : (h + 1) * 64],
                    op0=mybir.AluOpType.mult,
                    op1=mybir.AluOpType.add,
                )
                done_cnt[blk] = done_cnt.get(blk, 0) + 1
                if done_cnt[blk] == 4:
                    nc.sync.dma_start(
                        out[blk * 128 : (blk + 1) * 128, :], out_tiles[blk][:]
                    )
            return f

        # ---- attention pipeline (scores/exp decoupled from AV by <=3) ----
        blocks = [(p, sc) for p in range(2) for sc in range(NSC)]
        NB = len(blocks)
        ctx_ps = {}
        pts = {}

        def scores_exp(i):
            b, t = divmod(i, NTT)
            pair, sc = blocks[b]
            s0 = sc * 512
            qTt, kTt = qT[pair], kT[pair]
            tsl = slice(t * 128, (t + 1) * 128)
            g = pps.tile([128, 1024], F32, name="g", tag="grp", bufs=2)
            nc.tensor.matmul(
                g[:, 0:512],
                kTt[0:64, tsl],
                qTt[0:64, s0 : s0 + 512],
                start=True,
                stop=True,
                tile_position=(0, 0),
            )
            nc.tensor.matmul(
                g[:, 512:1024],
                kTt[64:128, tsl],
                qTt[64:128, s0 : s0 + 512],
                start=True,
                stop=True,
                tile_position=(64, 0),
            )
            pT_ = pTp.tile([128, 1024], BF16, name="pT_", tag="pT")
            nc.scalar.activation(
                pT_[:], g[:], mybir.ActivationFunctionType.Exp, scale=0.125
            )
            pts[i] = pT_

        def av(j):
            b, t = divmod(j, NTT)
            pair, sc = blocks[b]
            hA, hB = 2 * pair, 2 * pair + 1
            if t == 0:
                ctxA = pps.tile([65, 512], F32, name="ctxA", tag="ctx", bufs=2)
                ctxB = pps.tile([65, 512], F32, name="ctxB", tag="ctx", bufs=2)
                ctx_ps[b] = (ctxA, ctxB)
            ctxA, ctxB = ctx_ps[b]
            pT_ = pts.pop(j)
            st, sp = (t == 0), (t == NTT - 1)
            nc.tensor.matmul(
                ctxA[:], v_ext[t][:, hA, :], pT_[:, 0:512], start=st, stop=sp
            )
            nc.tensor.matmul(
                ctxB[:], v_ext[t][:, hB, :], pT_[:, 512:1024], start=st, stop=sp
            )
            if t == NTT - 1:
                del ctx_ps[b]
                csA = cxp.tile([65, 512], BF16, name="csA", tag="cs")
                nc.vector.tensor_copy(csA[:], ctxA[:])
                csB = cxp.tile([65, 512], BF16, name="csB", tag="cs")
                nc.vector.tensor_copy(csB[:], ctxB[:])
                for pi in range(4):
                    pieces.append(piece(csA, sc, hA, pi))
                    pieces.append(piece(csB, sc, hB, pi))

        # ---- ramp: stream chunks 0-3; v fills slot between early steps ----
        fill_kq("k", 0, 0)
        fill_kq("q", 0, 0)
        scores_exp(0)
        scores_exp(1)
        fill_v(0, 0)
        scores_exp(2)
        av(0)
        fill_v(0, 1)
        scores_exp(3)
        av(1)
        av(2)
        fill_kq("k", 0, 1)
        scores_exp(4)
        av(3)
        fill_v(1, 0)
        scores_exp(5)
        av(4)
        fill_v(1, 1)
        scores_exp(6)
        av(5)
        fill_kq("k", 0, 2)
        scores_exp(7)
        av(6)
        fill_v(2, 0)
        scores_exp(8)
        av(7)
        fill_v(2, 1)
        scores_exp(9)
        av(8)
        fill_kq("k", 0, 3)
        scores_exp(10)
        av(9)
        fill_v(3, 0)
        scores_exp(11)
        av(10)
        fill_v(3, 1)
        scores_exp(12)
        av(11)
        fill_kq("q", 0, 1)  # needed at step 16 (block 0,sc1)
        scores_exp(13)
        av(12)
        scores_exp(14)
        av(13)
        scores_exp(15)
        av(14)

        # ---- steady state: k-granular fills in deadline order ----
        units = deque()
        for kind, pair, c in [
            ("q", 0, 2),  # deadline step 32
            ("q", 0, 3),  # 48
            ("k", 1, 0),  # 64
            ("q", 1, 0),  # 64
            ("k", 1, 1),  # 68
            ("k", 1, 2),  # 72
            ("k", 1, 3),  # 76
            ("q", 1, 1),  # 80
            ("q", 1, 2),  # 96
            ("q", 1, 3),  # 112
        ]:
            units.extend(fill_kq_units(kind, pair, c))
            units.append("gap")  # piece window between fills

        for i in range(16, NB * NTT):
            scores_exp(i)
            av(i - 1)
            budget = 2 if i % 3 == 0 else 1
            for _ in range(budget):
                if units:
                    u = units.popleft()
                    if u == "gap":
                        for _ in range(2):
                            if pieces:
                                pieces.popleft()()
                    else:
                        u()
                elif pieces:
                    pieces.popleft()()
                    if len(pieces) > 8:
                        pieces.popleft()()
        av(NB * NTT - 1)
        while units:
            u = units.popleft()
            if u != "gap":
                u()
        while pieces:
            pieces.popleft()()

    nc.compile()
    return nc


def kernel(Q, Wq, bq, Wk, bk, Wv, bv):
    global _cached_nc, last_result
    Q = np.asarray(Q, dtype=np.float32)
    Wq, Wk, Wv = (np.asarray(w, dtype=np.float32) for w in (Wq, Wk, Wv))
    bq, bk, bv = (np.asarray(b, dtype=np.float32) for b in (bq, bk, bv))
    B = Q.shape[0]
    assert Q.shape == (B, S, D) and B * 4 == N_CORES

    if _cached_nc is None:
        _cached_nc = _build()
    nc = _cached_nc

    # host-side shard prep (bf16)
    bf = ml_dtypes.bfloat16
    qts = [np.ascontiguousarray(Q[b].T).astype(bf) for b in range(B)]
    wqs = [np.ascontiguousarray(Wq[g * J : (g + 1) * J, :].T).astype(bf) for g in range(4)]
    wks = [np.ascontiguousarray(Wk[g * J : (g + 1) * J, :].T).astype(bf) for g in range(4)]
    wvs = [np.ascontiguousarray(Wv[g * J : (g + 1) * J, :].T).astype(bf) for g in range(4)]

    in_maps = []
    for c in range(N_CORES):
        b, g = c // 4, c % 4
        jsl = slice(g * J, (g + 1) * J)
        in_maps.append(
            {
                "qt": qts[b],
                "wq": wqs[g],
                "wk": wks[g],
                "wv": wvs[g],
                "bq": np.ascontiguousarray(bq[jsl]),
                "bk": np.ascontiguousarray(bk[jsl]),
                "bv": np.ascontiguousarray(bv[jsl]),
            }
        )

    last_result = run_bass_kernel_spmd(nc, in_maps, list(range(N_CORES)))

    full = np.empty((B, S, D), dtype=np.float32)
    for c in range(N_CORES):
        b, g = c // 4, c % 4
        full[b, :, g * J : (g + 1) * J] = last_result.results[c]["out"]
    return full


# revision 26
# speedup vs baseline: 1.2225x; 1.0173x over previous
"""Multi-head self-attention Trainium2 kernel (8 NeuronCores, SPMD).

Problem: B=2, S=2048, D=1024, H=16, Dk=64; torch-style Linear projections
(x @ W.T + b), custom softmax: p = exp(scores/8), attn = p / (sum(p) + 1e-8).

Sharding: 32 (batch, head) pairs over 8 cores -> core c handles batch c//4,
heads [4*(c%4), 4*(c%4)+4). Each core projects only its 256 features of
q/k/v; attention is embarrassingly parallel over (b, h).

v3 structure (single continuous pipeline, bf16 data everywhere off-PSUM):
  - qt arrives in bf16 (halved DMA), streamed in 4 s-chunks of 512; all bulk
    DMA rides the sync HWDGE queue in dependency order (wk, qt0, wq, wv,
    qt1-3), biases on gpsimd, and the scalar engine issues NO DMA (its
    DMA_DIRECT2D retires only on transfer completion, delaying the exp
    stream ~10us). 16 dummy identity matmuls warm the PE's HAM clock gate
    during the DMA wait so the first projections run at 2.4GHz, not the
    1.2GHz mid-pstate (measured 2x on the first ~7us of fills).
  - as soon as chunk 0 is projected (kT[0] tiles 0-3, qT[0] chunk 0), the
    scores/exp stream starts; v fills and later chunks interleave between
    steps, with AV matmuls lagging up to 3 steps behind their exp (pT pool
    bufs=4) so v production never gates the exp stream.
  - attention steady state is ACT(exp)-bound (~1.11us per t-step, one
    [128,1024] exp covers both heads of a pair via tile_position-packed
    scoresT in a 2-bank PSUM tile); measured steady exp gap 1127ns.
  - remaining projection work (qT[0] chunks 2-3, pair-1 qT/kT) runs as
    k-granular units (one 512-col matmul each, ~230ns) paced ~1.33/step in
    deadline order, so a fill never inserts a >2us bubble into the PE
    stream. Finalize pieces (PE transpose -> reciprocal -> out=ctx*r+bv)
    run 2-at-a-time between fills (never while a fill holds an x-slot:
    the shared 2-slot PSUM rotation would deadlock the in-order PE).
  - PSUM budget: scores 2x[128,1024] (8KB/part) + ctx 2x[65,512] (4KB) +
    2 rotating fill/transpose slots [128,512] (4KB) = 16KB = all 8 banks.

Output per core: [2048, 256] fp32 -> host concatenates features per batch.
"""

import sys

sys.path.insert(0, "/opt/trn_rl_repo")

from collections import deque
from contextlib import ExitStack

import numpy as np
import ml_dtypes

import concourse.bass as bass
import concourse.tile as tile
from concourse import bacc, mybir
from concourse.bass_utils import run_bass_kernel_spmd
from concourse.masks import make_identity

F32 = mybir.dt.float32
F32R = mybir.dt.float32r
INT32 = mybir.dt.int32
INT16 = mybir.dt.int16
BF16 = mybir.dt.bfloat16

S = 2048  # sequence length
D = 1024  # d_model
J = 256  # features per core (4 heads x 64)
NKT = 8  # k-tiles of the d_model contraction
NSC = 4  # s-chunks of 512
NTT = 16  # t-tiles of 128
N_CORES = 8

_cached_nc = None
last_result = None  # BassKernelResults of the most recent run (for test.py)


def _build():
    nc = bacc.Bacc(None, target_bir_lowering=False)

    qt = nc.dram_tensor("qt", [D, S], BF16, kind="ExternalInput")
    wq = nc.dram_tensor("wq", [D, J], BF16, kind="ExternalInput")
    wk = nc.dram_tensor("wk", [D, J], BF16, kind="ExternalInput")
    wv = nc.dram_tensor("wv", [D, J], BF16, kind="ExternalInput")
    bq = nc.dram_tensor("bq", [J], F32, kind="ExternalInput")
    bk = nc.dram_tensor("bk", [J], F32, kind="ExternalInput")
    bv = nc.dram_tensor("bv", [J], F32, kind="ExternalInput")
    out = nc.dram_tensor("out", [S, J], F32, kind="ExternalOutput")

    with tile.TileContext(nc) as tc, ExitStack() as ctx:
        wts = ctx.enter_context(tc.tile_pool(name="wts", bufs=1))
        qtcp = ctx.enter_context(tc.tile_pool(name="qtc", bufs=1))
        qkp = ctx.enter_context(tc.tile_pool(name="qkp", bufs=1))
        vxp = ctx.enter_context(tc.tile_pool(name="vxp", bufs=1))
        bp = ctx.enter_context(tc.tile_pool(name="bp", bufs=1))
        cxp = ctx.enter_context(tc.tile_pool(name="cxp", bufs=16))
        pTp = ctx.enter_context(tc.tile_pool(name="pTp", bufs=4))
        outp = ctx.enter_context(tc.tile_pool(name="outp", bufs=1))
        rp = ctx.enter_context(tc.tile_pool(name="rp", bufs=8))
        ytp = ctx.enter_context(tc.tile_pool(name="ytp", bufs=2))
        pps = ctx.enter_context(tc.tile_pool(name="pps", bufs=1, space="PSUM"))

        # ---- DMA kickoff ----
        # scratch + identity first on gpsimd (gate the exp preload/warm-up)
        scratch = bp.tile([128, 1], F32, name="scratch")
        nc.gpsimd.memset(scratch[:], 0.0)
        ident = bp.tile([128, 128], BF16, name="ident")
        make_identity(nc, ident[:])

        # exp-table preload FIRST on ACT: a DMA_DIRECT2D on the scalar
        # engine retires only when its transfer completes, so any qt DMA
        # issued before the preload pushes the first real exp out ~10us
        nc.scalar.activation(
            scratch[:], scratch[:], mybir.ActivationFunctionType.Exp, scale=0.0
        )

        bq_t = bp.tile([128, 2], F32, name="bqt")
        bk_t = bp.tile([128, 2], F32, name="bkt")
        bv_t = bp.tile([128, J], F32, name="bvt")
        nc.gpsimd.dma_start(bq_t[:], bq.rearrange("(m p) -> p m", p=128))
        nc.gpsimd.dma_start(bk_t[:], bk.rearrange("(m p) -> p m", p=128))
        bvap = bv[:]
        bv_bcast = bass.AP(
            tensor=bvap.tensor, offset=bvap.offset, ap=[[0, 128], [1, J]]
        )
        nc.gpsimd.dma_start(bv_t[:], bv_bcast)

        # all bulk DMA rides the sync HWDGE queue in dependency order; the
        # scalar engine issues nothing (a DMA_DIRECT2D there would delay
        # the exp stream: it retires only when its transfer completes, and
        # the queue spin-up counts from the first issue)
        wk_t = [
            wts.tile([128, NKT, 128], BF16, name=f"wkt{m}", tag=f"wkt{m}")
            for m in range(2)
        ]
        wq_t = [
            wts.tile([128, NKT, 128], BF16, name=f"wqt{m}", tag=f"wqt{m}")
            for m in range(2)
        ]
        wv_t = wts.tile([128, NKT, J], BF16, name="wvt", tag="wvt")
        qt_r = qt.rearrange("(k p) s -> p k s", p=128)
        wk_r = wk.rearrange("(k p) j -> p k j", p=128)
        wq_r = wq.rearrange("(k p) j -> p k j", p=128)
        qtc = [
            qtcp.tile([128, NKT, 512], BF16, name=f"qtc{c}", tag=f"qtc{c}")
            for c in range(NSC)
        ]
        # qtc0 first (it gates pk the longest), then pair-0 weights, so the
        # first projections start ~2us earlier; pair-1 weights follow
        nc.sync.dma_start(qtc[0][:], qt_r[:, :, 0:512])
        nc.sync.dma_start(wk_t[0][:], wk_r[:, :, 0:128])
        nc.sync.dma_start(wq_t[0][:], wq_r[:, :, 0:128])
        nc.sync.dma_start(wk_t[1][:], wk_r[:, :, 128:256])
        nc.sync.dma_start(wq_t[1][:], wq_r[:, :, 128:256])
        nc.sync.dma_start(wv_t[:], wv.rearrange("(k p) j -> p k j", p=128))
        nc.sync.dma_start(qtc[1][:], qt_r[:, :, 512:1024])
        nc.sync.dma_start(qtc[2][:], qt_r[:, :, 1024:1536])
        nc.sync.dma_start(qtc[3][:], qt_r[:, :, 1536:2048])

        # Dummy matmuls keep the PE continuously busy from ~6.5us until
        # qtc0 lands (~13.7us), ramping the HAM clock gate to 2.4GHz before
        # the first projection fills. 16 ended at ~10.1us and the >3.4us
        # idle until the DMA-gated first fill re-throttled the PE to the
        # 1.2GHz mid-pstate (measured: 634ns/512col fills, ~6us lost).
        warm = pps.tile([128, 128], F32, name="warm", tag="x0")
        for _ in range(52):
            nc.tensor.matmul(warm[:], ident[:], ident[:], start=True, stop=True)

        # Persistent projected tensors (all bf16)
        qT = [qkp.tile([128, S], BF16, name=f"qT{m}", tag=f"qT{m}") for m in range(2)]
        kT = [qkp.tile([128, S], BF16, name=f"kT{m}", tag=f"kT{m}") for m in range(2)]
        v_ext = []
        for t in range(NTT):
            vt = vxp.tile([128, 4, 65], BF16, name=f"vx{t}", tag=f"vx{t}")
            nc.gpsimd.memset(vt[:], 1.0)  # ones col [:, h, 64] survives
            v_ext.append(vt)
        out_tiles = [
            outp.tile([128, J], F32, name=f"ot{b}", tag=f"ot{b}") for b in range(16)
        ]

        # ---- fill units (projection work, runs in the x0/x1 PSUM slots) ----
        xflip = [0]

        def xtag():
            tag = f"x{xflip[0] % 2}"
            xflip[0] += 1
            return tag

        def fill_kq(kind, pair, c):
            """Project 512 s-cols of qT/kT for one head pair from qtc[c]."""
            for u in fill_kq_units(kind, pair, c):
                u()

        def fill_kq_units(kind, pair, c):
            w_t, dst, b_t = {
                "k": (wk_t, kT, bk_t),
                "q": (wq_t, qT, bq_t),
            }[kind]
            cell = {}

            def mk(k):
                def f():
                    if k == 0:
                        cell["px"] = pps.tile(
                            [128, 512], F32, name=f"p{kind}", tag=xtag()
                        )
                    nc.tensor.matmul(
                        cell["px"][:],
                        w_t[pair][:, k, :],
                        qtc[c][:, k, :],
                        start=(k == 0),
                        stop=(k == NKT - 1),
                    )
                return f

            def cp():
                s0 = c * 512
                nc.vector.tensor_scalar_add(
                    dst[pair][:, s0 : s0 + 512],
                    cell["px"][:],
                    b_t[:, pair : pair + 1],
                )

            return [mk(k) for k in range(NKT)] + [cp]

        def fill_v(c, half):
            """Project 2 t-tiles x all 4 heads of v from qtc[c] into v_ext."""
            px = pps.tile([128, 512], F32, name="pv", tag=xtag())
            for ii in range(2):
                i = 2 * half + ii
                isl = slice(i * 128, (i + 1) * 128)
                for k in range(NKT):
                    nc.tensor.matmul(
                        px[:, ii * 256 : (ii + 1) * 256],
                        qtc[c][:, k, isl],
                        wv_t[:, k, :],
                        start=(k == 0),
                        stop=(k == NKT - 1),
                    )
            for ii in range(2):
                i = 2 * half + ii
                nc.vector.tensor_copy(
                    v_ext[c * 4 + i][:, :, 0:64],
                    px[:, ii * 256 : (ii + 1) * 256].rearrange(
                        "p (h d) -> p h d", h=4
                    ),
                )

        # ---- finalize pieces ----
        pieces = deque()
        done_cnt = {}

        def piece(cs_tile, sc, h, i):
            def f():
                tp = pps.tile([128, 65], BF16, name="tp", tag=xtag())
                nc.tensor.transpose(
                    tp[:],
                    cs_tile[0:65, i * 128 : (i + 1) * 128],
                    ident[0:65, 0:65],
                )
                r = rp.tile([128, 1], F32, name="r", tag="r")
                nc.vector.reciprocal(r[:], tp[:, 64:65])
                blk = sc * 4 + i
                nc.vector.scalar_tensor_tensor(
                    out=out_tiles[blk][:, h * 64 : (h + 1) * 64],
                    in0=tp[:, 0:64],
                    scalar=r[:],
                    in1=bv_t[:, h * 64 : (h + 1) * 64],
                    op0=mybir.AluOpType.mult,
                    op1=mybir.AluOpType.add,
                )
                done_cnt[blk] = done_cnt.get(blk, 0) + 1
                if done_cnt[blk] == 4:
                    nc.sync.dma_start(
                        out[blk * 128 : (blk + 1) * 128, :], out_tiles[blk][:]
                    )
            return f

        # ---- attention pipeline (scores/exp decoupled from AV by <=3) ----
        blocks = [(p, sc) for p in range(2) for sc in range(NSC)]
        NB = len(blocks)
        ctx_ps = {}
        pts = {}

        def scores_exp(i):
            b, t = divmod(i, NTT)
            pair, sc = blocks[b]
            s0 = sc * 512
            qTt, kTt = qT[pair], kT[pair]
            tsl = slice(t * 128, (t + 1) * 128)
            g = pps.tile([128, 1024], F32, name="g", tag="grp", bufs=2)
            nc.tensor.matmul(
                g[:, 0:512],
                kTt[0:64, tsl],
                qTt[0:64, s0 : s0 + 512],
                start=True,
                stop=True,
                tile_position=(0, 0),
            )
            nc.tensor.matmul(
                g[:, 512:1024],
                kTt[64:128, tsl],
                qTt[64:128, s0 : s0 + 512],
                start=True,
                stop=True,
                tile_position=(64, 0),
            )
            pT_ = pTp.tile([128, 1024], BF16, name="pT_", tag="pT")
            if False:
                # bf16 Schraudolph exp on DVE: i16 = g*(2^7*log2e/8) +
                # (127<<7 - C' + .5); the int16 bit pattern IS bf16 exp(g/8)
                # to ~3.4% (C'=5.59 centers the periodic interp error, which
                # the sum-normalized softmax largely cancels; sim rel ~1e-2
                # vs the 2e-2 gate). Offloading every 3rd steady step moves
                # ~37 of 128 exps off the ACT critical path onto idle DVE.
                nc.vector.tensor_scalar(
                    pT_[:].bitcast(INT16),
                    g[:],
                    23.083118654391137,
                    16250.909312,
                    mybir.AluOpType.mult,
                    mybir.AluOpType.add,
                )
            else:
                nc.scalar.activation(
                    pT_[:], g[:], mybir.ActivationFunctionType.Exp, scale=0.125
                )
            pts[i] = pT_

        def av(j):
            b, t = divmod(j, NTT)
            pair, sc = blocks[b]
            hA, hB = 2 * pair, 2 * pair + 1
            if t == 0:
                ctxA = pps.tile([65, 512], F32, name="ctxA", tag="ctx", bufs=2)
                ctxB = pps.tile([65, 512], F32, name="ctxB", tag="ctx", bufs=2)
                ctx_ps[b] = (ctxA, ctxB)
            ctxA, ctxB = ctx_ps[b]
            pT_ = pts.pop(j)
            st, sp = (t == 0), (t == NTT - 1)
            nc.tensor.matmul(
                ctxA[:], v_ext[t][:, hA, :], pT_[:, 0:512], start=st, stop=sp
            )
            nc.tensor.matmul(
                ctxB[:], v_ext[t][:, hB, :], pT_[:, 512:1024], start=st, stop=sp
            )
            if t == NTT - 1:
                del ctx_ps[b]
                csA = cxp.tile([65, 512], BF16, name="csA", tag="cs")
                nc.vector.tensor_copy(csA[:], ctxA[:])
                csB = cxp.tile([65, 512], BF16, name="csB", tag="cs")
                nc.vector.tensor_copy(csB[:], ctxB[:])
                for pi in range(4):
                    pieces.append(piece(csA, sc, hA, pi))
                    pieces.append(piece(csB, sc, hB, pi))

        # ---- ramp: stream chunks 0-3; v fills slot between early steps ----
        fill_kq("k", 0, 0)
        fill_kq("q", 0, 0)
        scores_exp(0)
        scores_exp(1)
        fill_v(0, 0)
        scores_exp(2)
        av(0)
        fill_v(0, 1)
        scores_exp(3)
        av(1)
        av(2)
        fill_kq("k", 0, 1)
        scores_exp(4)
        av(3)
        fill_v(1, 0)
        scores_exp(5)
        av(4)
        fill_v(1, 1)
        scores_exp(6)
        av(5)
        fill_kq("k", 0, 2)
        scores_exp(7)
        av(6)
        fill_v(2, 0)
        scores_exp(8)
        av(7)
        fill_v(2, 1)
        scores_exp(9)
        av(8)
        fill_kq("k", 0, 3)
        scores_exp(10)
        av(9)
        fill_v(3, 0)
        scores_exp(11)
        av(10)
        fill_v(3, 1)
        scores_exp(12)
        av(11)
        fill_kq("q", 0, 1)  # needed at step 16 (block 0,sc1)
        scores_exp(13)
        av(12)
        scores_exp(14)
        av(13)
        scores_exp(15)
        av(14)

        # ---- steady state: k-granular fills in deadline order ----
        units = deque()
        for kind, pair, c in [
            ("q", 0, 2),  # deadline step 32
            ("q", 0, 3),  # 48
            ("k", 1, 0),  # 64
            ("q", 1, 0),  # 64
            ("k", 1, 1),  # 68
            ("k", 1, 2),  # 72
            ("k", 1, 3),  # 76
            ("q", 1, 1),  # 80
            ("q", 1, 2),  # 96
            ("q", 1, 3),  # 112
        ]:
            units.extend(fill_kq_units(kind, pair, c))
            units.append("gap")  # piece window between fills

        for i in range(16, NB * NTT):
            scores_exp(i)
            av(i - 1)
            # block boundary (i%16 in (0,1)): av(i-1) just queued the cs
            # copies that gate the ctx rotation for this block's AVs; emit
            # no DVE work (pieces/copies) these 2 steps so they clear at
            # once. Fill units (pure PE matmuls) still pace at >=1/step so
            # projections never slip past their consumer blocks.
            boundary = i % 16 in (0, 1)
            budget = 1 if boundary else (2 if i % 3 == 0 else 1)
            for _ in range(budget):
                if units:
                    if boundary and units[0] == "gap":
                        break
                    u = units.popleft()
                    if u == "gap":
                        for _ in range(2):
                            if pieces:
                                pieces.popleft()()
                    else:
                        u()
                elif pieces and not boundary:
                    pieces.popleft()()
                    if len(pieces) > 8:
                        pieces.popleft()()
        av(NB * NTT - 1)
        while units:
            u = units.popleft()
            if u != "gap":
                u()
        while pieces:
            pieces.popleft()()

    nc.compile()
    return nc


def kernel(Q, Wq, bq, Wk, bk, Wv, bv):
    global _cached_nc, last_result
    Q = np.asarray(Q, dtype=np.float32)
    Wq, Wk, Wv = (np.asarray(w, dtype=np.float32) for w in (Wq, Wk, Wv))
    bq, bk, bv = (np.asarray(b, dtype=np.float32) for b in (bq, bk, bv))
    B = Q.shape[0]
    assert Q.shape == (B, S, D) and B * 4 == N_CORES

    if _cached_nc is None:
        _cached_nc = _build()
    nc = _cached_nc

    # host-side shard prep (bf16)
    bf = ml_dtypes.bfloat16
    qts = [np.ascontiguousarray(Q[b].T).astype(bf) for b in range(B)]
    wqs = [np.ascontiguousarray(Wq[g * J : (g + 1) * J, :].T).astype(bf) for g in range(4)]
    wks = [np.ascontiguousarray(Wk[g * J : (g + 1) * J, :].T).astype(bf) for g in range(4)]
    wvs = [np.ascontiguousarray(Wv[g * J : (g + 1) * J, :].T).astype(bf) for g in range(4)]

    in_maps = []
    for c in range(N_CORES):
        b, g = c // 4, c % 4
        jsl = slice(g * J, (g + 1) * J)
        in_maps.append(
            {
                "qt": qts[b],
                "wq": wqs[g],
                "wk": wks[g],
                "wv": wvs[g],
                "bq": np.ascontiguousarray(bq[jsl]),
                "bk": np.ascontiguousarray(bk[jsl]),
                "bv": np.ascontiguousarray(bv[jsl]),
            }
        )

    last_result = run_bass_kernel_spmd(nc, in_maps, list(range(N_CORES)))

    full = np.empty((B, S, D), dtype=np.float32)
    for c in range(N_CORES):
        b, g = c // 4, c % 4
        full[b, :, g * J : (g + 1) * J] = last_result.results[c]["out"]
    return full


# revision 27
# speedup vs baseline: 1.2232x; 1.0006x over previous
"""Multi-head self-attention Trainium2 kernel (8 NeuronCores, SPMD).

Problem: B=2, S=2048, D=1024, H=16, Dk=64; torch-style Linear projections
(x @ W.T + b), custom softmax: p = exp(scores/8), attn = p / (sum(p) + 1e-8).

Sharding: 32 (batch, head) pairs over 8 cores -> core c handles batch c//4,
heads [4*(c%4), 4*(c%4)+4). Each core projects only its 256 features of
q/k/v; attention is embarrassingly parallel over (b, h).

v3 structure (single continuous pipeline, bf16 data everywhere off-PSUM):
  - qt arrives in bf16 (halved DMA), streamed in 4 s-chunks of 512; all bulk
    DMA rides the sync HWDGE queue in dependency order (wk, qt0, wq, wv,
    qt1-3), biases on gpsimd, and the scalar engine issues NO DMA (its
    DMA_DIRECT2D retires only on transfer completion, delaying the exp
    stream ~10us). 16 dummy identity matmuls warm the PE's HAM clock gate
    during the DMA wait so the first projections run at 2.4GHz, not the
    1.2GHz mid-pstate (measured 2x on the first ~7us of fills).
  - as soon as chunk 0 is projected (kT[0] tiles 0-3, qT[0] chunk 0), the
    scores/exp stream starts; v fills and later chunks interleave between
    steps, with AV matmuls lagging up to 3 steps behind their exp (pT pool
    bufs=4) so v production never gates the exp stream.
  - attention steady state is ACT(exp)-bound (~1.11us per t-step, one
    [128,1024] exp covers both heads of a pair via tile_position-packed
    scoresT in a 2-bank PSUM tile); measured steady exp gap 1127ns.
  - remaining projection work (qT[0] chunks 2-3, pair-1 qT/kT) runs as
    k-granular units (one 512-col matmul each, ~230ns) paced ~1.33/step in
    deadline order, so a fill never inserts a >2us bubble into the PE
    stream. Finalize pieces (PE transpose -> reciprocal -> out=ctx*r+bv)
    run 2-at-a-time between fills (never while a fill holds an x-slot:
    the shared 2-slot PSUM rotation would deadlock the in-order PE).
  - PSUM budget: scores 2x[128,1024] (8KB/part) + ctx 2x[65,512] (4KB) +
    2 rotating fill/transpose slots [128,512] (4KB) = 16KB = all 8 banks.

Output per core: [2048, 256] fp32 -> host concatenates features per batch.
"""

import sys

sys.path.insert(0, "/opt/trn_rl_repo")

from collections import deque
from contextlib import ExitStack

import numpy as np
import ml_dtypes

import concourse.bass as bass
import concourse.tile as tile
from concourse import bacc, mybir
from concourse.bass_utils import run_bass_kernel_spmd
from concourse.masks import make_identity

F32 = mybir.dt.float32
F32R = mybir.dt.float32r
INT32 = mybir.dt.int32
INT16 = mybir.dt.int16
BF16 = mybir.dt.bfloat16

S = 2048  # sequence length
D = 1024  # d_model
J = 256  # features per core (4 heads x 64)
NKT = 8  # k-tiles of the d_model contraction
NSC = 4  # s-chunks of 512
NTT = 16  # t-tiles of 128
N_CORES = 8

_cached_nc = None
last_result = None  # BassKernelResults of the most recent run (for test.py)


def _build():
    nc = bacc.Bacc(None, target_bir_lowering=False)

    qt = nc.dram_tensor("qt", [D, S], BF16, kind="ExternalInput")
    wq = nc.dram_tensor("wq", [D, J], BF16, kind="ExternalInput")
    wk = nc.dram_tensor("wk", [D, J], BF16, kind="ExternalInput")
    wv = nc.dram_tensor("wv", [D, J], BF16, kind="ExternalInput")
    bq = nc.dram_tensor("bq", [J], F32, kind="ExternalInput")
    bk = nc.dram_tensor("bk", [J], F32, kind="ExternalInput")
    bv = nc.dram_tensor("bv", [J], F32, kind="ExternalInput")
    out = nc.dram_tensor("out", [S, J], F32, kind="ExternalOutput")

    with tile.TileContext(nc) as tc, ExitStack() as ctx:
        wts = ctx.enter_context(tc.tile_pool(name="wts", bufs=1))
        qtcp = ctx.enter_context(tc.tile_pool(name="qtc", bufs=1))
        qkp = ctx.enter_context(tc.tile_pool(name="qkp", bufs=1))
        vxp = ctx.enter_context(tc.tile_pool(name="vxp", bufs=1))
        bp = ctx.enter_context(tc.tile_pool(name="bp", bufs=1))
        cxp = ctx.enter_context(tc.tile_pool(name="cxp", bufs=16))
        pTp = ctx.enter_context(tc.tile_pool(name="pTp", bufs=6))
        outp = ctx.enter_context(tc.tile_pool(name="outp", bufs=1))
        rp = ctx.enter_context(tc.tile_pool(name="rp", bufs=8))
        ytp = ctx.enter_context(tc.tile_pool(name="ytp", bufs=2))
        pps = ctx.enter_context(tc.tile_pool(name="pps", bufs=1, space="PSUM"))

        # ---- DMA kickoff ----
        # scratch + identity first on gpsimd (gate the exp preload/warm-up)
        scratch = bp.tile([128, 1], F32, name="scratch")
        nc.gpsimd.memset(scratch[:], 0.0)
        ident = bp.tile([128, 128], BF16, name="ident")
        make_identity(nc, ident[:])

        # exp-table preload FIRST on ACT: a DMA_DIRECT2D on the scalar
        # engine retires only when its transfer completes, so any qt DMA
        # issued before the preload pushes the first real exp out ~10us
        nc.scalar.activation(
            scratch[:], scratch[:], mybir.ActivationFunctionType.Exp, scale=0.0
        )

        bq_t = bp.tile([128, 2], F32, name="bqt")
        bk_t = bp.tile([128, 2], F32, name="bkt")
        bv_t = bp.tile([128, J], F32, name="bvt")
        nc.gpsimd.dma_start(bq_t[:], bq.rearrange("(m p) -> p m", p=128))
        nc.gpsimd.dma_start(bk_t[:], bk.rearrange("(m p) -> p m", p=128))
        bvap = bv[:]
        bv_bcast = bass.AP(
            tensor=bvap.tensor, offset=bvap.offset, ap=[[0, 128], [1, J]]
        )
        nc.gpsimd.dma_start(bv_t[:], bv_bcast)

        # all bulk DMA rides the sync HWDGE queue in dependency order; the
        # scalar engine issues nothing (a DMA_DIRECT2D there would delay
        # the exp stream: it retires only when its transfer completes, and
        # the queue spin-up counts from the first issue)
        wk_t = [
            wts.tile([128, NKT, 128], BF16, name=f"wkt{m}", tag=f"wkt{m}")
            for m in range(2)
        ]
        wq_t = [
            wts.tile([128, NKT, 128], BF16, name=f"wqt{m}", tag=f"wqt{m}")
            for m in range(2)
        ]
        wv_t = wts.tile([128, NKT, J], BF16, name="wvt", tag="wvt")
        qt_r = qt.rearrange("(k p) s -> p k s", p=128)
        wk_r = wk.rearrange("(k p) j -> p k j", p=128)
        wq_r = wq.rearrange("(k p) j -> p k j", p=128)
        qtc = [
            qtcp.tile([128, NKT, 512], BF16, name=f"qtc{c}", tag=f"qtc{c}")
            for c in range(NSC)
        ]
        # qtc0 first (it gates pk the longest), then pair-0 weights, so the
        # first projections start ~2us earlier; pair-1 weights follow
        nc.sync.dma_start(qtc[0][:], qt_r[:, :, 0:512])
        nc.sync.dma_start(wk_t[0][:], wk_r[:, :, 0:128])
        nc.sync.dma_start(wq_t[0][:], wq_r[:, :, 0:128])
        nc.sync.dma_start(wk_t[1][:], wk_r[:, :, 128:256])
        nc.sync.dma_start(wq_t[1][:], wq_r[:, :, 128:256])
        nc.sync.dma_start(wv_t[:], wv.rearrange("(k p) j -> p k j", p=128))
        nc.sync.dma_start(qtc[1][:], qt_r[:, :, 512:1024])
        nc.sync.dma_start(qtc[2][:], qt_r[:, :, 1024:1536])
        nc.sync.dma_start(qtc[3][:], qt_r[:, :, 1536:2048])

        # Dummy matmuls keep the PE continuously busy from ~6.5us until
        # qtc0 lands (~13.7us), ramping the HAM clock gate to 2.4GHz before
        # the first projection fills. 16 ended at ~10.1us and the >3.4us
        # idle until the DMA-gated first fill re-throttled the PE to the
        # 1.2GHz mid-pstate (measured: 634ns/512col fills, ~6us lost).
        warm = pps.tile([128, 128], F32, name="warm", tag="x0")
        for _ in range(52):
            nc.tensor.matmul(warm[:], ident[:], ident[:], start=True, stop=True)

        # Persistent projected tensors (all bf16)
        qT = [qkp.tile([128, S], BF16, name=f"qT{m}", tag=f"qT{m}") for m in range(2)]
        kT = [qkp.tile([128, S], BF16, name=f"kT{m}", tag=f"kT{m}") for m in range(2)]
        v_ext = []
        for t in range(NTT):
            vt = vxp.tile([128, 4, 65], BF16, name=f"vx{t}", tag=f"vx{t}")
            nc.gpsimd.memset(vt[:], 1.0)  # ones col [:, h, 64] survives
            v_ext.append(vt)
        out_tiles = [
            outp.tile([128, J], F32, name=f"ot{b}", tag=f"ot{b}") for b in range(16)
        ]

        # ---- fill units (projection work, runs in the x0/x1 PSUM slots) ----
        xflip = [0]

        def xtag():
            tag = f"x{xflip[0] % 2}"
            xflip[0] += 1
            return tag

        def fill_kq(kind, pair, c):
            """Project 512 s-cols of qT/kT for one head pair from qtc[c]."""
            for u in fill_kq_units(kind, pair, c):
                u()

        def fill_kq_units(kind, pair, c):
            w_t, dst, b_t = {
                "k": (wk_t, kT, bk_t),
                "q": (wq_t, qT, bq_t),
            }[kind]
            cell = {}

            def mk(k):
                def f():
                    if k == 0:
                        cell["px"] = pps.tile(
                            [128, 512], F32, name=f"p{kind}", tag=xtag()
                        )
                    nc.tensor.matmul(
                        cell["px"][:],
                        w_t[pair][:, k, :],
                        qtc[c][:, k, :],
                        start=(k == 0),
                        stop=(k == NKT - 1),
                    )
                return f

            def cp():
                s0 = c * 512
                nc.vector.tensor_scalar_add(
                    dst[pair][:, s0 : s0 + 512],
                    cell["px"][:],
                    b_t[:, pair : pair + 1],
                )

            return [mk(k) for k in range(NKT)] + [cp]

        def fill_v(c, half):
            """Project 2 t-tiles x all 4 heads of v from qtc[c] into v_ext."""
            px = pps.tile([128, 512], F32, name="pv", tag=xtag())
            for ii in range(2):
                i = 2 * half + ii
                isl = slice(i * 128, (i + 1) * 128)
                for k in range(NKT):
                    nc.tensor.matmul(
                        px[:, ii * 256 : (ii + 1) * 256],
                        qtc[c][:, k, isl],
                        wv_t[:, k, :],
                        start=(k == 0),
                        stop=(k == NKT - 1),
                    )
            for ii in range(2):
                i = 2 * half + ii
                nc.vector.tensor_copy(
                    v_ext[c * 4 + i][:, :, 0:64],
                    px[:, ii * 256 : (ii + 1) * 256].rearrange(
                        "p (h d) -> p h d", h=4
                    ),
                )

        # ---- finalize pieces ----
        pieces = deque()
        done_cnt = {}

        def piece(cs_tile, sc, h, i):
            def f():
                tp = pps.tile([128, 65], BF16, name="tp", tag=xtag())
                nc.tensor.transpose(
                    tp[:],
                    cs_tile[0:65, i * 128 : (i + 1) * 128],
                    ident[0:65, 0:65],
                )
                r = rp.tile([128, 1], F32, name="r", tag="r")
                nc.vector.reciprocal(r[:], tp[:, 64:65])
                blk = sc * 4 + i
                nc.vector.scalar_tensor_tensor(
                    out=out_tiles[blk][:, h * 64 : (h + 1) * 64],
                    in0=tp[:, 0:64],
                    scalar=r[:],
                    in1=bv_t[:, h * 64 : (h + 1) * 64],
                    op0=mybir.AluOpType.mult,
                    op1=mybir.AluOpType.add,
                )
                done_cnt[blk] = done_cnt.get(blk, 0) + 1
                if done_cnt[blk] == 4:
                    nc.sync.dma_start(
                        out[blk * 128 : (blk + 1) * 128, :], out_tiles[blk][:]
                    )
            return f

        # ---- attention pipeline (scores/exp decoupled from AV by <=3) ----
        blocks = [(p, sc) for p in range(2) for sc in range(NSC)]
        NB = len(blocks)
        ctx_ps = {}
        pts = {}

        def scores_exp(i):
            b, t = divmod(i, NTT)
            pair, sc = blocks[b]
            s0 = sc * 512
            qTt, kTt = qT[pair], kT[pair]
            tsl = slice(t * 128, (t + 1) * 128)
            g = pps.tile([128, 1024], F32, name="g", tag="grp", bufs=2)
            nc.tensor.matmul(
                g[:, 0:512],
                kTt[0:64, tsl],
                qTt[0:64, s0 : s0 + 512],
                start=True,
                stop=True,
                tile_position=(0, 0),
            )
            nc.tensor.matmul(
                g[:, 512:1024],
                kTt[64:128, tsl],
                qTt[64:128, s0 : s0 + 512],
                start=True,
                stop=True,
                tile_position=(64, 0),
            )
            pT_ = pTp.tile([128, 1024], BF16, name="pT_", tag="pT")
            if False:
                # bf16 Schraudolph exp on DVE: i16 = g*(2^7*log2e/8) +
                # (127<<7 - C' + .5); the int16 bit pattern IS bf16 exp(g/8)
                # to ~3.4% (C'=5.59 centers the periodic interp error, which
                # the sum-normalized softmax largely cancels; sim rel ~1e-2
                # vs the 2e-2 gate). Offloading every 3rd steady step moves
                # ~37 of 128 exps off the ACT critical path onto idle DVE.
                nc.vector.tensor_scalar(
                    pT_[:].bitcast(INT16),
                    g[:],
                    23.083118654391137,
                    16250.909312,
                    mybir.AluOpType.mult,
                    mybir.AluOpType.add,
                )
            else:
                nc.scalar.activation(
                    pT_[:], g[:], mybir.ActivationFunctionType.Exp, scale=0.125
                )
            pts[i] = pT_

        def av(j):
            b, t = divmod(j, NTT)
            pair, sc = blocks[b]
            hA, hB = 2 * pair, 2 * pair + 1
            if t == 0:
                ctxA = pps.tile([65, 512], F32, name="ctxA", tag="ctx", bufs=2)
                ctxB = pps.tile([65, 512], F32, name="ctxB", tag="ctx", bufs=2)
                ctx_ps[b] = (ctxA, ctxB)
            ctxA, ctxB = ctx_ps[b]
            pT_ = pts.pop(j)
            st, sp = (t == 0), (t == NTT - 1)
            nc.tensor.matmul(
                ctxA[:], v_ext[t][:, hA, :], pT_[:, 0:512], start=st, stop=sp
            )
            nc.tensor.matmul(
                ctxB[:], v_ext[t][:, hB, :], pT_[:, 512:1024], start=st, stop=sp
            )
            if t == NTT - 1:
                del ctx_ps[b]
                csA = cxp.tile([65, 512], BF16, name="csA", tag="cs")
                nc.vector.tensor_copy(csA[:], ctxA[:])
                csB = cxp.tile([65, 512], BF16, name="csB", tag="cs")
                nc.vector.tensor_copy(csB[:], ctxB[:])
                for pi in range(4):
                    pieces.append(piece(csA, sc, hA, pi))
                    pieces.append(piece(csB, sc, hB, pi))

        # ---- ramp: stream chunks 0-3; v fills slot between early steps ----
        fill_kq("k", 0, 0)
        fill_kq("q", 0, 0)
        scores_exp(0)
        scores_exp(1)
        fill_v(0, 0)
        scores_exp(2)
        av(0)
        fill_v(0, 1)
        scores_exp(3)
        av(1)
        av(2)
        fill_kq("k", 0, 1)
        scores_exp(4)
        av(3)
        fill_v(1, 0)
        scores_exp(5)
        av(4)
        fill_v(1, 1)
        scores_exp(6)
        av(5)
        fill_kq("k", 0, 2)
        scores_exp(7)
        av(6)
        fill_v(2, 0)
        scores_exp(8)
        av(7)
        fill_v(2, 1)
        scores_exp(9)
        av(8)
        fill_kq("k", 0, 3)
        scores_exp(10)
        av(9)
        fill_v(3, 0)
        scores_exp(11)
        av(10)
        fill_v(3, 1)
        scores_exp(12)
        av(11)
        fill_kq("q", 0, 1)  # needed at step 16 (block 0,sc1)
        scores_exp(13)
        av(12)
        scores_exp(14)
        av(13)
        scores_exp(15)
        av(14)

        # ---- steady state: k-granular fills in deadline order ----
        units = deque()
        for kind, pair, c in [
            ("q", 0, 2),  # deadline step 32
            ("q", 0, 3),  # 48
            ("k", 1, 0),  # 64
            ("q", 1, 0),  # 64
            ("k", 1, 1),  # 68
            ("k", 1, 2),  # 72
            ("k", 1, 3),  # 76
            ("q", 1, 1),  # 80
            ("q", 1, 2),  # 96
            ("q", 1, 3),  # 112
        ]:
            units.extend(fill_kq_units(kind, pair, c))
            units.append("gap")  # piece window between fills

        for i in range(16, NB * NTT):
            scores_exp(i)
            av(i - 1)
            # block boundary (i%16 in (0,1)): av(i-1) just queued the cs
            # copies that gate the ctx rotation for this block's AVs; emit
            # no DVE work (pieces/copies) these 2 steps so they clear at
            # once. Fill units (pure PE matmuls) still pace at >=1/step so
            # projections never slip past their consumer blocks.
            boundary = i % 16 in (0, 1)
            budget = 1 if boundary else (2 if i % 3 == 0 else 1)
            for _ in range(budget):
                if units:
                    if boundary and units[0] == "gap":
                        break
                    u = units.popleft()
                    if u == "gap":
                        for _ in range(2):
                            if pieces:
                                pieces.popleft()()
                    else:
                        u()
                elif pieces and not boundary:
                    pieces.popleft()()
                    if len(pieces) > 8:
                        pieces.popleft()()
        av(NB * NTT - 1)
        while units:
            u = units.popleft()
            if u != "gap":
                u()
        while pieces:
            pieces.popleft()()

    nc.compile()
    return nc


def kernel(Q, Wq, bq, Wk, bk, Wv, bv):
    global _cached_nc, last_result
    Q = np.asarray(Q, dtype=np.float32)
    Wq, Wk, Wv = (np.asarray(w, dtype=np.float32) for w in (Wq, Wk, Wv))
    bq, bk, bv = (np.asarray(b, dtype=np.float32) for b in (bq, bk, bv))
    B = Q.shape[0]
    assert Q.shape == (B, S, D) and B * 4 == N_CORES

    if _cached_nc is None:
        _cached_nc = _build()
    nc = _cached_nc

    # host-side shard prep (bf16)
    bf = ml_dtypes.bfloat16
    qts = [np.ascontiguousarray(Q[b].T).astype(bf) for b in range(B)]
    wqs = [np.ascontiguousarray(Wq[g * J : (g + 1) * J, :].T).astype(bf) for g in range(4)]
    wks = [np.ascontiguousarray(Wk[g * J : (g + 1) * J, :].T).astype(bf) for g in range(4)]
    wvs = [np.ascontiguousarray(Wv[g * J : (g + 1) * J, :].T).astype(bf) for g in range(4)]

    in_maps = []
    for c in range(N_CORES):
        b, g = c // 4, c % 4
        jsl = slice(g * J, (g + 1) * J)
        in_maps.append(
            {
                "qt": qts[b],
                "wq": wqs[g],
                "wk": wks[g],
                "wv": wvs[g],
                "bq": np.ascontiguousarray(bq[jsl]),
                "bk": np.ascontiguousarray(bk[jsl]),
                "bv": np.ascontiguousarray(bv[jsl]),
            }
        )

    last_result = run_bass_kernel_spmd(nc, in_maps, list(range(N_CORES)))

    full = np.empty((B, S, D), dtype=np.float32)
    for c in range(N_CORES):
        b, g = c // 4, c % 4
        full[b, :, g * J : (g + 1) * J] = last_result.results[c]["out"]
    return full
